# revision 11
# baseline (speedup 1.0000x reference)
"""FATM (wavelet spiking module) Trainium2 Bass kernel — v2.

Data-parallel over B across 8 NeuronCores (B=8 -> 1 sample/core).

v2 layout/op changes vs v1 (same validated layout algebra):
  - x shipped as f16 (0.5*x); u1 LIF state f16 -> LIF1 runs entirely in
    DVE 2x/4x perf modes (tensor_scalar/stt all-f16 SBUF unit-stride).
  - LIF1 hard reset via is_lt mask + multiply (no copy_predicated f32).
  - Stream transposes batched over all T=4 steps (one [128,4096] ST per
    stage instead of four [128,1024]).
  - negIF2 state lives in PSUM: mix/P0 matmuls accumulate straight into
    it; soft reset via -0.5*I matmul feedback (like negIF1). Kills the
    v2sb DVE add/reset and the MX evacuation.
  - bn0/LIF2 params materialized as full [128,1024] f16 tiles (DRAM
    consts) -> unit-stride operands, no stride-0 broadcast APs.
  - identity (+x) enters OUT PSUM via a 2*I matmul on f16 x; betaA bias
    applied by the ACT evacuation (bias AP); output DMA'd as f16.
  - Phased emission per q: LIF1(all t) -> ST1 -> colMM/evac(all t) ->
    ST2 -> per-t negIF1/LIF2 -> per-t invrow/evac -> ST3 -> invcol/evac
    -> ST4 -> per-t mix+negIF2 + conv/OUT/evac/DMA. PSUM: V1+v2 states
    (2x2 banks) + 2x[128,1024] scratch = 8 banks.
"""
import os
import sys
sys.path.insert(0, '/opt/trn_rl_repo')
sys.path.insert(0, '/root/.axon_site/_ro/trn_rl_repo')

import numpy as np

import bass_rust
from concourse import bass, mybir
import concourse.tile as tile_mod
from concourse.tile import TileContext
from concourse.vector_clock import ScopedClock
from concourse.bass_utils import run_bass_kernel_spmd

# ------------------------------------------------------------- walrus fix
MAX_WAITS = 1


def _patched_drain_and_barrier(self, tick_clock, wait_clock):
    drain_inst = self.nc.sync.drain()
    wait_clock.add_sem_waits(
        drain_inst.ins, ScopedClock({None: tick_clock.global_clock})
    )
    si = drain_inst.ins.sync_info
    if si is not None and si.on_wait and len(si.on_wait) > MAX_WAITS:
        waits = list(si.on_wait)
        si.on_wait = waits[:MAX_WAITS]
        for i in range(MAX_WAITS, len(waits), MAX_WAITS):
            nop = self.nc.sync.nop(nofuse=True, hint="wait_spill")
            nop.ins.sync_info = bass_rust.SyncInfo(
                on_wait=waits[i:i + MAX_WAITS], on_update=[]
            )
    self.nc.all_engine_barrier()
    assert self.sems is not None
    popped = self.nc._tile_sem_poison_stack.pop()
    assert popped is self._sem_poison
    self.nc.clear_and_free_semaphores(list(self.sems.allocated().values()))
    self.nc.all_engine_barrier()


tile_mod.TileContext._drain_and_barrier = _patched_drain_and_barrier


def _split_excess_waits(nc):
    """This walrus build rejects >1 sync wait per instruction; spill excess
    waits onto same-engine nops inserted before the instruction."""
    n_split = 0
    for bb in nc.main_func.blocks:
        insts = list(bb.instructions)
        out, changed = [], False
        for ins in insts:
            si = ins.sync_info
            if si is not None and si.on_wait and len(si.on_wait) > MAX_WAITS:
                waits = list(si.on_wait)
                si.on_wait = waits[-MAX_WAITS:]
                for i in range(0, len(waits) - MAX_WAITS, MAX_WAITS):
                    nop = mybir.InstNoOp(name=f"{ins.name}_wsp{i}", ins=[],
                                         outs=[])
                    nop.engine = ins.engine
                    nop.sync_info = bass_rust.SyncInfo(
                        on_wait=waits[i:i + MAX_WAITS], on_update=[])
                    out.append(nop)
                    n_split += 1
                changed = True
            out.append(ins)
        if changed:
            try:
                bb.instructions = out
            except Exception:
                lst = bb.instructions
                lst.clear()
                lst.extend(out)
    return n_split


# ---------------------------------------------------------------- consts
EPS = 1e-5
T, Bb, C, Hh, Ww = 4, 8, 512, 32, 32
NQ, HW = 4, 1024
THW = T * HW
NCORES = 8
F32 = mybir.dt.float32
F16 = mybir.dt.float16
U16 = mybir.dt.uint16
ALU = mybir.AluOpType
AF = mybir.ActivationFunctionType
TAPS = [(dy, dx) for dy in (-1, 0, 1) for dx in (-1, 0, 1)]

# f16 [128, x] consts packed into one DRAM array (order defines offsets)
PACK128 = [
    ('lfwdT', 128), ('linvT', 128), ('negIT', 128), ('id2', 128),
    ('mixT', NQ * 128), ('convT', NQ * 9 * 128), ('a2dT', NQ * 128),
    ('inv0cF', NQ * HW), ('thr2F', NQ * HW), ('negBF', NQ * HW),
]
PACK1 = [('p0row', HW), ('beta1', NQ * 128)]


def _haar_matrix(n):
    h = np.array([[1.0]])
    while h.shape[0] < n:
        top = np.kron(h, [1.0, 1.0])
        bot = np.kron(np.eye(h.shape[0]), [1.0, -1.0])
        h = np.concatenate([top, bot], axis=0) / np.sqrt(2.0)
    return h


def _bd4(block_fn):
    L = np.zeros((128, 128))
    for cb in range(4):
        L[32 * cb:32 * cb + 32, 32 * cb:32 * cb + 32] = block_fn(cb)
    return L


def _host_consts(inputs):
    hw_ = np.asarray(inputs['haar_weight'], np.float64)
    w1 = np.asarray(inputs['conv1_w'], np.float64)[:, :, 0, 0]
    b1 = np.asarray(inputs['conv1_b'], np.float64)
    w2 = np.asarray(inputs['conv2_w'], np.float64)
    b2 = np.asarray(inputs['conv2_b'], np.float64)
    bnw = np.asarray(inputs['bn_weight'], np.float64)
    bnb = np.asarray(inputs['bn_bias'], np.float64)
    bnm = np.asarray(inputs['bn_mean'], np.float64)
    bnv = np.asarray(inputs['bn_var'], np.float64)
    inv = bnw / np.sqrt(bnv + EPS)
    bbias = bnb - bnm * inv

    Q = _haar_matrix(32)
    P0flat = (Q.T @ np.ones((32, 32)) @ Q).reshape(HW)

    d = {}
    d['lfwdT'] = _bd4(lambda cb: Q.T)        # [p=(cb,w), m=(cb,l)] = Q[l,w]
    d['linvT'] = _bd4(lambda cb: Q)          # [p=(cb,i), m=(cb,a)] = Q[i,a]
    d['negIT'] = -0.5 * np.eye(128)
    d['id2'] = 2.0 * np.eye(128)
    d['p0row'] = P0flat.reshape(1, HW)

    mixT = np.zeros((NQ, 128, 128))
    convT = np.zeros((NQ, 9, 128, 128))
    a2dT = np.zeros((NQ, 128, 128))
    beta1 = np.zeros((NQ, 128))
    betaA = np.zeros((NQ, 128))
    inv0cF = np.zeros((NQ, 128, HW))
    thr2F = np.zeros((NQ, 128, HW))
    negBF = np.zeros((NQ, 128, HW))

    for q in range(NQ):
        cidx = np.arange(128 * q, 128 * q + 128)
        inv0q, bias0q = inv[0][cidx], bbias[0][cidx]
        inv1q, bias1q = inv[1][cidx], bbias[1][cidx]
        inv2q, bias2q = inv[2][cidx], bbias[2][cidx]
        inv3q, bias3q = inv[3][cidx], bbias[3][cidx]
        inv4q, bias4q = inv[4][cidx], bbias[4][cidx]

        mixT[q] = _bd4(
            lambda cb: hw_[4 * q + cb] * inv1q.reshape(4, 32)[cb][None, :])
        for ti, (dy, dx) in enumerate(TAPS):
            def cb_blk(cb, dy=dy, dx=dx):
                m = w2[:, :, dy + 1, dx + 1].T * \
                    inv4q.reshape(4, 32)[cb][None, :]
                if dy == 0 and dx == 0:
                    m = m + w1.T * inv3q.reshape(4, 32)[cb][None, :]
                return m
            convT[q, ti] = _bd4(cb_blk)
        a2dT[q] = np.diag(inv2q / 2.0)
        beta1[q] = bias1q
        betaA[q] = (inv4q * np.tile(b2, 16)[cidx] + bias4q
                    + inv3q * np.tile(b1, 16)[cidx] + bias3q + bias2q)

        # full [128, (cc,l)] param tiles in the transformed layout:
        # value at (p=(cb,i), f=(cc*32+l)) = param[q, cb, cc]
        def full_tile(v):          # v: [128] per-channel (cb,cc)
            blk = v.reshape(4, 32)                      # [cb, cc]
            t_ = np.repeat(blk[:, None, :], 32, axis=1)  # [cb, i, cc]
            t_ = t_.reshape(128, 32)                    # [p, cc]
            return np.repeat(t_[:, :, None], 32, axis=2).reshape(128, HW)

        inv0cF[q] = full_tile(inv0q / 4.0)
        thr2F[q] = full_tile(1.0 - bias0q)
        negBF[q] = full_tile(-bias0q)

    d['mixT'] = mixT.transpose(1, 0, 2).reshape(128, NQ * 128)
    d['convT'] = convT.transpose(2, 0, 1, 3).reshape(128, NQ * 9 * 128)
    d['a2dT'] = a2dT.transpose(1, 0, 2).reshape(128, NQ * 128)
    d['inv0cF'] = inv0cF.transpose(1, 0, 2).reshape(128, NQ * HW)
    d['thr2F'] = thr2F.transpose(1, 0, 2).reshape(128, NQ * HW)
    d['negBF'] = negBF.transpose(1, 0, 2).reshape(128, NQ * HW)
    d['beta1'] = beta1.reshape(1, NQ * 128)

    c128 = np.concatenate(
        [np.asarray(d[n]).reshape(128, w) for n, w in PACK128],
        axis=1).astype(np.float16)
    c1 = np.concatenate(
        [np.asarray(d[n]).reshape(1, w) for n, w in PACK1],
        axis=1).astype(np.float16)
    betaAc = np.ascontiguousarray(betaA.T).astype(np.float32)  # [128, NQ]
    return np.ascontiguousarray(c128), np.ascontiguousarray(c1), betaAc


P128_W = sum(w for _, w in PACK128)
P1_W = sum(w for _, w in PACK1)
P128_OFF = {}
_o = 0
for _n, _w in PACK128:
    P128_OFF[_n] = (_o, _w)
    _o += _w
P1_OFF = {}
_o = 0
for _n, _w in PACK1:
    P1_OFF[_n] = (_o, _w)
    _o += _w


def _build_program():
    nc = bass.Bass("TRN2", target_bir_lowering=False, debug=False)
    x16d = nc.declare_dram_parameter("x16h", [NQ, 128, THW], F16,
                                     isOutput=False)
    c128d = nc.declare_dram_parameter("c128", [128, P128_W], F16,
                                      isOutput=False)
    c1d = nc.declare_dram_parameter("c1", [1, P1_W], F16, isOutput=False)
    betaAd = nc.declare_dram_parameter("betaA", [128, NQ], F32,
                                       isOutput=False)
    outd = nc.declare_dram_parameter("out", [NQ, 128, THW], F16,
                                     isOutput=True)

    with TileContext(nc) as tc:
        with (
            tc.tile_pool(name="consts", bufs=1) as cpool,
            tc.tile_pool(name="xq", bufs=2) as xqpool,
            tc.tile_pool(name="big", bufs=1) as bigpool,
            tc.tile_pool(name="state", bufs=1) as spool,
            tc.tile_pool(name="spp", bufs=7) as sppool,
            tc.tile_pool(name="wk", bufs=3) as wpool,
            tc.tile_pool(name="ost", bufs=3) as ostpool,
            tc.tile_pool(name="psSt", bufs=1, space="PSUM") as psSt,
            tc.tile_pool(name="psScr", bufs=2, space="PSUM") as psScr,
        ):
            c128 = cpool.tile([128, P128_W], F16, tag="c128")
            nc.sync.dma_start(c128[:, :], c128d.ap())
            c1 = cpool.tile([1, P1_W], F16, tag="c1")
            nc.sync.dma_start(c1[:, :], c1d.ap())
            betaA = cpool.tile([128, NQ], F32, tag="betaA")
            nc.sync.dma_start(betaA[:, :], betaAd.ap())
            one_m1 = cpool.tile([128, 1], F32, tag="bm1")
            one_p1 = cpool.tile([128, 1], F32, tag="bp1")
            nc.vector.memset(one_m1[:, :], -1.0)
            nc.vector.memset(one_p1[:, :], 1.0)

            def c128s(name, idx=0, w=128):
                off, _ = P128_OFF[name]
                return c128[:, off + idx * w: off + (idx + 1) * w]

            def c1s(name, idx=0, w=None):
                off, tot = P1_OFF[name]
                if w is None:
                    w = tot
                return c1[0:1, off + idx * w: off + (idx + 1) * w]

            lfwdT = c128s('lfwdT')
            linvT = c128s('linvT')
            negIT = c128s('negIT')
            id2 = c128s('id2')

            # pre-fetch all xq tiles (double buffered)
            xqs = {}

            def fetch_x(q):
                xq = xqpool.tile([128, THW], F16, tag="xq")
                nc.sync.dma_start(xq[:, :], x16d.ap()[q])
                xqs[q] = xq

            fetch_x(0)

            for q in range(NQ):
                if q + 1 < NQ:
                    fetch_x(q + 1)
                xq = xqs[q]
                inv0cF = c128s('inv0cF', q, HW)
                thr2F = c128s('thr2F', q, HW)
                negBF = c128s('negBF', q, HW)

                # ---------------- phase A: LIF1 (all t) + fwd col ----
                u1h = spool.tile([128, HW], F16, tag="u1h")
                nc.vector.memset(u1h[:, :], 0.0)
                sN = bigpool.tile([128, THW], F16, tag="sN")
                sps = []
                for t in range(T):
                    xt = xq[:, t * HW:(t + 1) * HW]
                    nc.vector.scalar_tensor_tensor(
                        u1h[:, :], u1h[:, :], 0.5, xt, ALU.mult, ALU.add)
                    sp = sppool.tile([128, 34 * 34], F16, tag="sp")
                    if q == 0 or (q == 1 and t < 3):
                        # 7 pool bufs; interiors are always overwritten,
                        # borders must be zero once per buf (first 7 allocs)
                        nc.gpsimd.memset(sp[:, :], 0.0)
                    sp3 = sp[:, :].rearrange("p (h w) -> p h w", h=34, w=34)
                    nc.vector.tensor_scalar(
                        sp3[:, 1:33, 1:33],
                        u1h[:, :].rearrange("p (h w) -> p h w", h=32, w=32),
                        1.0, None, ALU.is_ge)
                    nc.vector.tensor_scalar(
                        sN[:, t * HW:(t + 1) * HW], u1h[:, :], 1.0, None,
                        ALU.is_ge)
                    ns = wpool.tile([128, HW], F16, tag="ns")
                    nc.vector.tensor_scalar(ns[:, :], u1h[:, :], 1.0, None,
                                            ALU.is_lt)
                    nc.vector.tensor_tensor(u1h[:, :], u1h[:, :], ns[:, :],
                                            ALU.mult)
                    sps.append(sp3)

                sD = bigpool.tile([128, THW], F16, tag="sD")
                nc.vector.transpose(sD[:, :], sN[:, :])

                M1s = bigpool.tile([128, THW], F16, tag="M1s")
                for t in range(T):
                    M1 = psScr.tile([128, HW], F32, tag="scr")
                    sD3 = sD[:, t * HW:(t + 1) * HW].rearrange(
                        "p (h cc) -> p h cc", h=32, cc=32)
                    for j in (0, 1):
                        nc.tensor.matmul(
                            M1[:, 512 * j:512 * j + 512].rearrange(
                                "p (cc h) -> p h cc", cc=16, h=32),
                            lfwdT, sD3[:, :, 16 * j:16 * j + 16],
                            start=True, stop=True)
                    nc.scalar.copy(M1s[:, t * HW:(t + 1) * HW], M1[:, :])

                sB = bigpool.tile([128, THW], F16, tag="sB")
                nc.vector.transpose(sB[:, :], M1s[:, :])

                # ---------------- phase B: negIF1 + bn0/LIF2 ---------
                V1 = psSt.tile([128, HW], F32, tag="V1")
                u2h = spool.tile([128, HW], F16, tag="u2h")
                nc.vector.tensor_copy(u2h[:, :], negBF)
                s2a = bigpool.tile([128, THW], F16, tag="s2a")

                for t in range(T):
                    for h_ in (0, 512):
                        nc.tensor.matmul(
                            V1[:, h_:h_ + 512], lfwdT,
                            sB[:, t * HW + h_:t * HW + h_ + 512],
                            start=(t == 0), stop=(t == T - 1),
                            skip_group_check=True)
                    g1 = wpool.tile([128, HW], F16, tag="g1")
                    g2 = wpool.tile([128, HW], F16, tag="g2")
                    nc.scalar.activation(g1[:, :], V1[:, :], AF.Sign,
                                         bias=one_m1[:, :])
                    nc.scalar.activation(g2[:, :], V1[:, :], AF.Sign,
                                         bias=one_p1[:, :])
                    st1 = wpool.tile([128, HW], F16, tag="st1")
                    nc.gpsimd.tensor_tensor(st1[:, :], g1[:, :], g2[:, :],
                                            ALU.add)
                    if t < T - 1:
                        for h_ in (0, 512):
                            nc.tensor.matmul(V1[:, h_:h_ + 512], negIT,
                                             st1[:, h_:h_ + 512],
                                             start=False, stop=False,
                                             skip_group_check=True)
                    pprod = wpool.tile([128, HW], F16, tag="pprod")
                    nc.vector.tensor_tensor(pprod[:, :], st1[:, :], inv0cF,
                                            ALU.mult)
                    nc.vector.scalar_tensor_tensor(
                        u2h[:, :], u2h[:, :], 0.5, pprod[:, :],
                        ALU.mult, ALU.add)
                    s2t = s2a[:, t * HW:(t + 1) * HW]
                    nc.vector.tensor_tensor(s2t, u2h[:, :], thr2F,
                                            ALU.is_ge)
                    nc.vector.copy_predicated(u2h[:, :], s2t.bitcast(U16),
                                              negBF)

                # ---------------- phase C: inverse haar --------------
                Zs = bigpool.tile([128, THW], F16, tag="Zs")
                for t in range(T):
                    Z = psScr.tile([128, HW], F32, tag="scr")
                    for h_ in (0, 512):
                        nc.tensor.matmul(
                            Z[:, h_:h_ + 512], linvT,
                            s2a[:, t * HW + h_:t * HW + h_ + 512],
                            start=True, stop=True)
                    nc.scalar.copy(Zs[:, t * HW:(t + 1) * HW], Z[:, :])

                ZT = bigpool.tile([128, THW], F16, tag="ZT")
                nc.vector.transpose(ZT[:, :], Zs[:, :])

                Ws = bigpool.tile([128, THW], F16, tag="Ws")
                for t in range(T):
                    W2 = psScr.tile([128, HW], F32, tag="scr")
                    ZT3 = ZT[:, t * HW:(t + 1) * HW].rearrange(
                        "p (cc a) -> p cc a", cc=32, a=32)
                    for j in (0, 1):
                        nc.tensor.matmul(
                            W2[:, 512 * j:512 * j + 512].rearrange(
                                "p (a cc) -> p cc a", a=16, cc=32),
                            linvT, ZT3[:, :, 16 * j:16 * j + 16],
                            start=True, stop=True)
                    nc.scalar.copy(Ws[:, t * HW:(t + 1) * HW], W2[:, :])

                haarA = bigpool.tile([128, THW], F16, tag="haarA")
                nc.vector.transpose(haarA[:, :], Ws[:, :])

                # -------- phase D: mix->negIF2 (PSUM) + conv/out -----
                v2 = psSt.tile([128, HW], F32, tag="v2")
                st2a = bigpool.tile([128, THW], F16, tag="st2a")
                mixTq = c128s('mixT', q)
                a2dTq = c128s('a2dT', q)

                for t in range(T):
                    for h_ in (0, 512):
                        nc.tensor.matmul(v2[:, h_:h_ + 512], mixTq,
                                         haarA[:, t * HW + h_:
                                               t * HW + h_ + 512],
                                         start=(t == 0), stop=False,
                                         skip_group_check=True)
                        nc.tensor.matmul(v2[:, h_:h_ + 512],
                                         c1s('beta1', q, 128),
                                         c1s('p0row')[0:1, h_:h_ + 512],
                                         start=False, stop=(t == T - 1),
                                         skip_group_check=True)
                    g1b = wpool.tile([128, HW], F16, tag="g1b")
                    g2b = wpool.tile([128, HW], F16, tag="g2b")
                    nc.scalar.activation(g1b[:, :], v2[:, :], AF.Sign,
                                         bias=one_m1[:, :])
                    nc.scalar.activation(g2b[:, :], v2[:, :], AF.Sign,
                                         bias=one_p1[:, :])
                    st2t = st2a[:, t * HW:(t + 1) * HW]
                    nc.gpsimd.tensor_tensor(st2t, g1b[:, :], g2b[:, :],
                                            ALU.add)
                    if t < T - 1:
                        for h_ in (0, 512):
                            nc.tensor.matmul(v2[:, h_:h_ + 512], negIT,
                                             st2a[:, t * HW + h_:
                                                  t * HW + h_ + 512],
                                             start=False, stop=False,
                                             skip_group_check=True)

                    # conv + a2d + identity into OUT psum
                    OUT = psScr.tile([128, HW], F32, tag="scr")
                    sp3 = sps[t]
                    for ti in range(9):
                        dy, dx = TAPS[ti]
                        cT = c128s('convT', q * 9 + ti)
                        rhs = sp3[:, 1 + dy:33 + dy, 1 + dx:33 + dx]
                        nc.tensor.matmul(OUT[:, 0:512], cT,
                                         rhs[:, 0:16, :],
                                         start=(ti == 0), stop=False,
                                         skip_group_check=True)
                        nc.tensor.matmul(OUT[:, 512:1024], cT,
                                         rhs[:, 16:32, :],
                                         start=(ti == 0), stop=False,
                                         skip_group_check=True)
                    for h_ in (0, 512):
                        nc.tensor.matmul(OUT[:, h_:h_ + 512], a2dTq,
                                         st2a[:, t * HW + h_:
                                              t * HW + h_ + 512],
                                         start=False, stop=False,
                                         skip_group_check=True)
                        nc.tensor.matmul(OUT[:, h_:h_ + 512], id2,
                                         xq[:, t * HW + h_:
                                            t * HW + h_ + 512],
                                         start=False, stop=True,
                                         skip_group_check=True)
                    osb = ostpool.tile([128, HW], F16, tag="ost")
                    nc.scalar.activation(osb[:, :], OUT[:, :], AF.Identity,
                                         bias=betaA[:, q:q + 1])
                    nc.sync.dma_start(
                        outd.ap()[q][:, t * HW:(t + 1) * HW], osb[:, :])

    _split_excess_waits(nc)
    return nc


_NC_CACHE = None


def _get_nc():
    global _NC_CACHE
    if _NC_CACHE is None:
        _NC_CACHE = _build_program()
    return _NC_CACHE


def _prep_inputs(inputs):
    x = np.asarray(inputs['x'], np.float32)          # [T, B, C, H, W]
    c128, c1, betaAc = _host_consts(inputs)
    in_maps = []
    for b in range(NCORES):
        # [T, C, HW] -> [NQ, 128, T*HW], halved for the LIF1 decay form
        xb = (0.5 * x[:, b]).reshape(T, NQ, 128, HW).transpose(1, 2, 0, 3)
        m = {'x16h': np.ascontiguousarray(xb).reshape(NQ, 128, THW)
             .astype(np.float16),
             'c128': c128, 'c1': c1, 'betaA': betaAc}
        in_maps.append(m)
    return in_maps


def kernel(**inputs):
    in_maps = _prep_inputs(inputs)
    nc = _get_nc()
    res = run_bass_kernel_spmd(nc, in_maps, list(range(NCORES))).results
    # out [NQ, 128, T*HW] f16 -> [T, B, C, H, W] f32
    outs = []
    for b in range(NCORES):
        ob = res[b]['out'].astype(np.float32).reshape(NQ, 128, T, HW) \
            .transpose(2, 0, 1, 3)
        outs.append(ob.reshape(T, C, HW))
    out = np.stack(outs, axis=1)
    return out.reshape(T, Bb, C, Hh, Ww).astype(np.float32)


# revision 17
# speedup vs baseline: 1.1255x; 1.1255x over previous
"""FATM (wavelet spiking module) Trainium2 Bass kernel — v2.

Data-parallel over B across 8 NeuronCores (B=8 -> 1 sample/core).

v2 layout/op changes vs v1 (same validated layout algebra):
  - x shipped as f16 (0.5*x); u1 LIF state f16 -> LIF1 runs entirely in
    DVE 2x/4x perf modes (tensor_scalar/stt all-f16 SBUF unit-stride).
  - LIF1 hard reset via is_lt mask + multiply (no copy_predicated f32).
  - Stream transposes batched over all T=4 steps (one [128,4096] ST per
    stage instead of four [128,1024]).
  - negIF2 state lives in PSUM: mix/P0 matmuls accumulate straight into
    it; soft reset via -0.5*I matmul feedback (like negIF1). Kills the
    v2sb DVE add/reset and the MX evacuation.
  - bn0/LIF2 params materialized as full [128,1024] f16 tiles (DRAM
    consts) -> unit-stride operands, no stride-0 broadcast APs.
  - identity (+x) enters OUT PSUM via a 2*I matmul on f16 x; betaA bias
    applied by the ACT evacuation (bias AP); output DMA'd as f16.
  - Phased emission per q: LIF1(all t) -> ST1 -> colMM/evac(all t) ->
    ST2 -> per-t negIF1/LIF2 -> per-t invrow/evac -> ST3 -> invcol/evac
    -> ST4 -> per-t mix+negIF2 + conv/OUT/evac/DMA. PSUM: V1+v2 states
    (2x2 banks) + 2x[128,1024] scratch = 8 banks.
"""
import os
import sys
sys.path.insert(0, '/opt/trn_rl_repo')
sys.path.insert(0, '/root/.axon_site/_ro/trn_rl_repo')

import numpy as np

import bass_rust
from concourse import bass, mybir
import concourse.tile as tile_mod
from concourse.tile import TileContext
from concourse.vector_clock import ScopedClock
from concourse.bass_utils import run_bass_kernel_spmd

# ------------------------------------------------------------- walrus fix
MAX_WAITS = 1


def _patched_drain_and_barrier(self, tick_clock, wait_clock):
    drain_inst = self.nc.sync.drain()
    wait_clock.add_sem_waits(
        drain_inst.ins, ScopedClock({None: tick_clock.global_clock})
    )
    si = drain_inst.ins.sync_info
    if si is not None and si.on_wait and len(si.on_wait) > MAX_WAITS:
        waits = list(si.on_wait)
        si.on_wait = waits[:MAX_WAITS]
        for i in range(MAX_WAITS, len(waits), MAX_WAITS):
            nop = self.nc.sync.nop(nofuse=True, hint="wait_spill")
            nop.ins.sync_info = bass_rust.SyncInfo(
                on_wait=waits[i:i + MAX_WAITS], on_update=[]
            )
    self.nc.all_engine_barrier()
    assert self.sems is not None
    popped = self.nc._tile_sem_poison_stack.pop()
    assert popped is self._sem_poison
    self.nc.clear_and_free_semaphores(list(self.sems.allocated().values()))
    self.nc.all_engine_barrier()


tile_mod.TileContext._drain_and_barrier = _patched_drain_and_barrier


def _split_excess_waits(nc):
    """This walrus build rejects >1 sync wait per instruction; spill excess
    waits onto same-engine nops inserted before the instruction."""
    n_split = 0
    for bb in nc.main_func.blocks:
        insts = list(bb.instructions)
        out, changed = [], False
        for ins in insts:
            si = ins.sync_info
            if si is not None and si.on_wait and len(si.on_wait) > MAX_WAITS:
                waits = list(si.on_wait)
                si.on_wait = waits[-MAX_WAITS:]
                for i in range(0, len(waits) - MAX_WAITS, MAX_WAITS):
                    nop = mybir.InstNoOp(name=f"{ins.name}_wsp{i}", ins=[],
                                         outs=[])
                    nop.engine = ins.engine
                    nop.sync_info = bass_rust.SyncInfo(
                        on_wait=waits[i:i + MAX_WAITS], on_update=[])
                    out.append(nop)
                    n_split += 1
                changed = True
            out.append(ins)
        if changed:
            try:
                bb.instructions = out
            except Exception:
                lst = bb.instructions
                lst.clear()
                lst.extend(out)
    return n_split


# ---------------------------------------------------------------- consts
EPS = 1e-5
T, Bb, C, Hh, Ww = 4, 8, 512, 32, 32
NQ, HW = 4, 1024
THW = T * HW
NCORES = 8
F32 = mybir.dt.float32
F16 = mybir.dt.float16
U16 = mybir.dt.uint16
ALU = mybir.AluOpType
AF = mybir.ActivationFunctionType
TAPS = [(dy, dx) for dy in (-1, 0, 1) for dx in (-1, 0, 1)]

# f16 [128, x] consts packed into one DRAM array (order defines offsets)
PACK128 = [
    ('lfwdT', 128), ('linvT', 128), ('negIT', 128), ('idI', T * 128),
    ('mixT', NQ * 128), ('convT', NQ * 9 * 128), ('a2dT', NQ * 128),
    ('inv0cF', NQ * HW), ('thr2F', NQ * HW), ('negBF', NQ * HW),
]
PACK1 = [('p0row', HW), ('beta1', NQ * 128)]


def _haar_matrix(n):
    h = np.array([[1.0]])
    while h.shape[0] < n:
        top = np.kron(h, [1.0, 1.0])
        bot = np.kron(np.eye(h.shape[0]), [1.0, -1.0])
        h = np.concatenate([top, bot], axis=0) / np.sqrt(2.0)
    return h


def _bd4(block_fn):
    L = np.zeros((128, 128))
    for cb in range(4):
        L[32 * cb:32 * cb + 32, 32 * cb:32 * cb + 32] = block_fn(cb)
    return L


def _host_consts(inputs):
    hw_ = np.asarray(inputs['haar_weight'], np.float64)
    w1 = np.asarray(inputs['conv1_w'], np.float64)[:, :, 0, 0]
    b1 = np.asarray(inputs['conv1_b'], np.float64)
    w2 = np.asarray(inputs['conv2_w'], np.float64)
    b2 = np.asarray(inputs['conv2_b'], np.float64)
    bnw = np.asarray(inputs['bn_weight'], np.float64)
    bnb = np.asarray(inputs['bn_bias'], np.float64)
    bnm = np.asarray(inputs['bn_mean'], np.float64)
    bnv = np.asarray(inputs['bn_var'], np.float64)
    inv = bnw / np.sqrt(bnv + EPS)
    bbias = bnb - bnm * inv

    Q = _haar_matrix(32)
    P0flat = (Q.T @ np.ones((32, 32)) @ Q).reshape(HW)

    d = {}
    d['lfwdT'] = _bd4(lambda cb: Q.T)        # [p=(cb,w), m=(cb,l)] = Q[l,w]
    d['linvT'] = _bd4(lambda cb: Q)          # [p=(cb,i), m=(cb,a)] = Q[i,a]
    d['negIT'] = -0.5 * np.eye(128)
    # identity-add weights per t: x16[t] = 2^(t-1)*x_t, so x_t = 2^(1-t)*x16
    d['idI'] = np.concatenate(
        [2.0 ** (1 - t) * np.eye(128) for t in range(T)], axis=1)
    d['p0row'] = P0flat.reshape(1, HW)

    mixT = np.zeros((NQ, 128, 128))
    convT = np.zeros((NQ, 9, 128, 128))
    a2dT = np.zeros((NQ, 128, 128))
    beta1 = np.zeros((NQ, 128))
    betaA = np.zeros((NQ, 128))
    inv0cF = np.zeros((NQ, 128, HW))
    thr2F = np.zeros((NQ, 128, HW))
    negBF = np.zeros((NQ, 128, HW))

    for q in range(NQ):
        cidx = np.arange(128 * q, 128 * q + 128)
        inv0q, bias0q = inv[0][cidx], bbias[0][cidx]
        inv1q, bias1q = inv[1][cidx], bbias[1][cidx]
        inv2q, bias2q = inv[2][cidx], bbias[2][cidx]
        inv3q, bias3q = inv[3][cidx], bbias[3][cidx]
        inv4q, bias4q = inv[4][cidx], bbias[4][cidx]

        mixT[q] = _bd4(
            lambda cb: hw_[4 * q + cb] * inv1q.reshape(4, 32)[cb][None, :])
        for ti, (dy, dx) in enumerate(TAPS):
            def cb_blk(cb, dy=dy, dx=dx):
                m = w2[:, :, dy + 1, dx + 1].T * \
                    inv4q.reshape(4, 32)[cb][None, :]
                if dy == 0 and dx == 0:
                    m = m + w1.T * inv3q.reshape(4, 32)[cb][None, :]
                return m
            convT[q, ti] = _bd4(cb_blk)
        a2dT[q] = np.diag(inv2q / 2.0)
        beta1[q] = bias1q
        betaA[q] = (inv4q * np.tile(b2, 16)[cidx] + bias4q
                    + inv3q * np.tile(b1, 16)[cidx] + bias3q + bias2q)

        # full [128, (cc,l)] param tiles in the transformed layout:
        # value at (p=(cb,i), f=(cc*32+l)) = param[q, cb, cc]
        def full_tile(v):          # v: [128] per-channel (cb,cc)
            blk = v.reshape(4, 32)                      # [cb, cc]
            t_ = np.repeat(blk[:, None, :], 32, axis=1)  # [cb, i, cc]
            t_ = t_.reshape(128, 32)                    # [p, cc]
            return np.repeat(t_[:, :, None], 32, axis=2).reshape(128, HW)

        inv0cF[q] = full_tile(inv0q / 4.0)
        thr2F[q] = full_tile(1.0 - bias0q)
        negBF[q] = full_tile(-bias0q)

    d['mixT'] = mixT.transpose(1, 0, 2).reshape(128, NQ * 128)
    d['convT'] = convT.transpose(2, 0, 1, 3).reshape(128, NQ * 9 * 128)
    d['a2dT'] = a2dT.transpose(1, 0, 2).reshape(128, NQ * 128)
    d['inv0cF'] = inv0cF.transpose(1, 0, 2).reshape(128, NQ * HW)
    d['thr2F'] = thr2F.transpose(1, 0, 2).reshape(128, NQ * HW)
    d['negBF'] = negBF.transpose(1, 0, 2).reshape(128, NQ * HW)
    d['beta1'] = beta1.reshape(1, NQ * 128)

    c128 = np.concatenate(
        [np.asarray(d[n]).reshape(128, w) for n, w in PACK128],
        axis=1).astype(np.float16)
    c1 = np.concatenate(
        [np.asarray(d[n]).reshape(1, w) for n, w in PACK1],
        axis=1).astype(np.float16)
    betaAc = np.ascontiguousarray(betaA.T).astype(np.float32)  # [128, NQ]
    return np.ascontiguousarray(c128), np.ascontiguousarray(c1), betaAc


P128_W = sum(w for _, w in PACK128)
P1_W = sum(w for _, w in PACK1)
P128_OFF = {}
_o = 0
for _n, _w in PACK128:
    P128_OFF[_n] = (_o, _w)
    _o += _w
P1_OFF = {}
_o = 0
for _n, _w in PACK1:
    P1_OFF[_n] = (_o, _w)
    _o += _w


def _build_program():
    nc = bass.Bass("TRN2", target_bir_lowering=False, debug=False)
    x16d = nc.declare_dram_parameter("x16h", [NQ, 128, THW], F16,
                                     isOutput=False)
    c128d = nc.declare_dram_parameter("c128", [128, P128_W], F16,
                                      isOutput=False)
    c1d = nc.declare_dram_parameter("c1", [1, P1_W], F16, isOutput=False)
    betaAd = nc.declare_dram_parameter("betaA", [128, NQ], F32,
                                       isOutput=False)
    outd = nc.declare_dram_parameter("out", [NQ, 128, THW], F16,
                                     isOutput=True)

    with TileContext(nc) as tc:
        with (
            tc.tile_pool(name="consts", bufs=1) as cpool,
            tc.tile_pool(name="xq", bufs=2) as xqpool,
            tc.tile_pool(name="big", bufs=1) as bigpool,
            tc.tile_pool(name="state", bufs=1) as spool,
            tc.tile_pool(name="spp", bufs=8) as sppool,
            tc.tile_pool(name="wk", bufs=2) as wpool,
            tc.tile_pool(name="ost", bufs=3) as ostpool,
            tc.tile_pool(name="psSt", bufs=1, space="PSUM") as psSt,
            tc.tile_pool(name="psScr", bufs=2, space="PSUM") as psScr,
        ):
            c128 = cpool.tile([128, P128_W], F16, tag="c128")
            nc.sync.dma_start(c128[:, :], c128d.ap())
            c1 = cpool.tile([1, P1_W], F16, tag="c1")
            nc.sync.dma_start(c1[:, :], c1d.ap())
            betaA = cpool.tile([128, NQ], F32, tag="betaA")
            nc.sync.dma_start(betaA[:, :], betaAd.ap())
            one_m1 = cpool.tile([128, 1], F32, tag="bm1")
            one_p1 = cpool.tile([128, 1], F32, tag="bp1")
            nc.vector.memset(one_m1[:, :], -1.0)
            nc.vector.memset(one_p1[:, :], 1.0)

            def c128s(name, idx=0, w=128):
                off, _ = P128_OFF[name]
                return c128[:, off + idx * w: off + (idx + 1) * w]

            def c1s(name, idx=0, w=None):
                off, tot = P1_OFF[name]
                if w is None:
                    w = tot
                return c1[0:1, off + idx * w: off + (idx + 1) * w]

            lfwdT = c128s('lfwdT')
            linvT = c128s('linvT')
            negIT = c128s('negIT')

            xqs = {}

            def fetch_x(q):
                xq = xqpool.tile([128, THW], F16, tag="xq")
                nc.sync.dma_start(xq[:, :], x16d.ap()[q])
                xqs[q] = xq

            # ---- per-q chunk builders; st holds per-q tiles ----
            def h1_chunks(q, st):
                """front half: LIF1 (scaled, leak-free), ST1, colMMs, ST2"""
                def pre():
                    fetch_x(q + 1) if q + 1 < NQ else None
                    st['xq'] = xqs[q]
                    st['u1h'] = spool.tile([128, HW], F16, tag="u1h", name=f"u1h{q}")
                    nc.vector.memset(st['u1h'][:, :], 0.0)
                    st['sN'] = bigpool.tile([128, THW], F16, tag="sN", name=f"sN{q}")
                    st['sps'] = []

                def lif1(t):
                    u1h = st['u1h']
                    thr = float(2.0 ** t)
                    nc.vector.tensor_tensor(
                        u1h[:, :], u1h[:, :],
                        st['xq'][:, t * HW:(t + 1) * HW], ALU.add)
                    sp = sppool.tile([128, 34 * 34], F16, tag="sp")
                    if q < 2:
                        # 8 pool bufs; interiors always overwritten,
                        # borders must be zero once per buf
                        nc.gpsimd.memset(sp[:, :], 0.0)
                    sp3 = sp[:, :].rearrange("p (h w) -> p h w", h=34,
                                             w=34)
                    nc.vector.tensor_scalar(
                        sp3[:, 1:33, 1:33],
                        u1h[:, :].rearrange("p (h w) -> p h w", h=32,
                                            w=32),
                        thr, None, ALU.is_ge)
                    nc.vector.tensor_scalar(
                        st['sN'][:, t * HW:(t + 1) * HW], u1h[:, :], thr,
                        None, ALU.is_ge)
                    ns = wpool.tile([128, HW], F16, tag="ns")
                    nc.vector.tensor_scalar(ns[:, :], u1h[:, :], thr,
                                            None, ALU.is_lt)
                    if t < T - 1:
                        nc.vector.tensor_tensor(u1h[:, :], u1h[:, :],
                                                ns[:, :], ALU.mult)
                    st['sps'].append(sp3)

                def st1ch():
                    st['sD'] = bigpool.tile([128, THW], F16, tag="sD", name=f"sD{q}")
                    nc.vector.transpose(st['sD'][:, :], st['sN'][:, :])
                    st['M1s'] = bigpool.tile([128, THW], F16, tag="M1s", name=f"M1s{q}")

                def colmm(t):
                    M1 = psScr.tile([128, HW], F32, tag="scr")
                    sD3 = st['sD'][:, t * HW:(t + 1) * HW].rearrange(
                        "p (h cc) -> p h cc", h=32, cc=32)
                    for j in (0, 1):
                        nc.tensor.matmul(
                            M1[:, 512 * j:512 * j + 512].rearrange(
                                "p (cc h) -> p h cc", cc=16, h=32),
                            lfwdT, sD3[:, :, 16 * j:16 * j + 16],
                            start=True, stop=True)
                    nc.scalar.copy(st['M1s'][:, t * HW:(t + 1) * HW],
                                   M1[:, :])

                def st2ch():
                    st['sB'] = bigpool.tile([128, THW], F16, tag="sB", name=f"sB{q}")
                    nc.vector.transpose(st['sB'][:, :], st['M1s'][:, :])

                def first(t=0):
                    pre()
                    lif1(0)
                return ([first] + [lambda t=t: lif1(t) for t in (1, 2, 3)]
                        + [st1ch]
                        + [lambda t=t: colmm(t) for t in range(T)]
                        + [st2ch])

            def h2_chunks(q, st):
                """back half: negIF1+LIF2, inverse haar, mix+negIF2+out"""
                inv0cF = c128s('inv0cF', q, HW)
                thr2F = c128s('thr2F', q, HW)
                negBF = c128s('negBF', q, HW)
                mixTq = c128s('mixT', q)
                a2dTq = c128s('a2dT', q)

                def bpre():
                    st['V1'] = psSt.tile([128, HW], F32, tag="V1", name=f"V1{q}")
                    st['u2h'] = spool.tile([128, HW], F16, tag="u2h", name=f"u2h{q}")
                    nc.vector.tensor_copy(st['u2h'][:, :], negBF)
                    st['s2a'] = bigpool.tile([128, THW], F16, tag="s2a", name=f"s2a{q}")

                def bt(t):
                    V1 = st['V1']
                    u2h = st['u2h']
                    for h_ in (0, 512):
                        nc.tensor.matmul(
                            V1[:, h_:h_ + 512], lfwdT,
                            st['sB'][:, t * HW + h_:t * HW + h_ + 512],
                            start=(t == 0), stop=(t == T - 1),
                            skip_group_check=True)
                    g1 = wpool.tile([128, HW], F16, tag="g1")
                    g2 = wpool.tile([128, HW], F16, tag="g2")
                    nc.scalar.activation(g1[:, :], V1[:, :], AF.Sign,
                                         bias=one_m1[:, :])
                    nc.scalar.activation(g2[:, :], V1[:, :], AF.Sign,
                                         bias=one_p1[:, :])
                    st1 = wpool.tile([128, HW], F16, tag="st1")
                    nc.gpsimd.tensor_tensor(st1[:, :], g1[:, :], g2[:, :],
                                            ALU.add)
                    if t < T - 1:
                        for h_ in (0, 512):
                            nc.tensor.matmul(V1[:, h_:h_ + 512], negIT,
                                             st1[:, h_:h_ + 512],
                                             start=False, stop=False,
                                             skip_group_check=True)
                    pprod = wpool.tile([128, HW], F16, tag="pprod")
                    nc.vector.tensor_tensor(pprod[:, :], st1[:, :],
                                            inv0cF, ALU.mult)
                    uh = wpool.tile([128, HW], F16, tag="uh")
                    nc.vector.tensor_scalar(uh[:, :], u2h[:, :], 0.5,
                                            None, ALU.mult)
                    nc.vector.tensor_tensor(u2h[:, :], uh[:, :],
                                            pprod[:, :], ALU.add)
                    s2t = st['s2a'][:, t * HW:(t + 1) * HW]
                    nc.vector.tensor_tensor(s2t, u2h[:, :], thr2F,
                                            ALU.is_ge)
                    nc.vector.copy_predicated(u2h[:, :],
                                              s2t.bitcast(U16), negBF)

                def zpre():
                    st['Zs'] = bigpool.tile([128, THW], F16, tag="Zs", name=f"Zs{q}")

                def zrow(t):
                    Z = psScr.tile([128, HW], F32, tag="scr")
                    for h_ in (0, 512):
                        nc.tensor.matmul(
                            Z[:, h_:h_ + 512], linvT,
                            st['s2a'][:, t * HW + h_:t * HW + h_ + 512],
                            start=True, stop=True)
                    nc.scalar.copy(st['Zs'][:, t * HW:(t + 1) * HW],
                                   Z[:, :])

                def st3ch():
                    st['ZT'] = bigpool.tile([128, THW], F16, tag="ZT", name=f"ZT{q}")
                    nc.vector.transpose(st['ZT'][:, :], st['Zs'][:, :])
                    st['Ws'] = bigpool.tile([128, THW], F16, tag="Ws", name=f"Ws{q}")

                def wcol(t):
                    W2 = psScr.tile([128, HW], F32, tag="scr")
                    ZT3 = st['ZT'][:, t * HW:(t + 1) * HW].rearrange(
                        "p (cc a) -> p cc a", cc=32, a=32)
                    for j in (0, 1):
                        nc.tensor.matmul(
                            W2[:, 512 * j:512 * j + 512].rearrange(
                                "p (a cc) -> p cc a", a=16, cc=32),
                            linvT, ZT3[:, :, 16 * j:16 * j + 16],
                            start=True, stop=True)
                    nc.scalar.copy(st['Ws'][:, t * HW:(t + 1) * HW],
                                   W2[:, :])

                def st4ch():
                    st['haarA'] = bigpool.tile([128, THW], F16, tag="haarA", name=f"haarA{q}")
                    nc.vector.transpose(st['haarA'][:, :], st['Ws'][:, :])
                    st['v2'] = psSt.tile([128, HW], F32, tag="v2", name=f"v2{q}")
                    st['st2a'] = bigpool.tile([128, THW], F16, tag="st2a", name=f"st2a{q}")

                def dt(t):
                    v2 = st['v2']
                    st2a = st['st2a']
                    for h_ in (0, 512):
                        nc.tensor.matmul(v2[:, h_:h_ + 512], mixTq,
                                         st['haarA'][:, t * HW + h_:
                                                     t * HW + h_ + 512],
                                         start=(t == 0), stop=False,
                                         skip_group_check=True)
                        nc.tensor.matmul(v2[:, h_:h_ + 512],
                                         c1s('beta1', q, 128),
                                         c1s('p0row')[0:1, h_:h_ + 512],
                                         start=False, stop=(t == T - 1),
                                         skip_group_check=True)
                    g1b = wpool.tile([128, HW], F16, tag="g1b")
                    g2b = wpool.tile([128, HW], F16, tag="g2b")
                    nc.scalar.activation(g1b[:, :], v2[:, :], AF.Sign,
                                         bias=one_m1[:, :])
                    nc.scalar.activation(g2b[:, :], v2[:, :], AF.Sign,
                                         bias=one_p1[:, :])
                    st2t = st2a[:, t * HW:(t + 1) * HW]
                    nc.gpsimd.tensor_tensor(st2t, g1b[:, :], g2b[:, :],
                                            ALU.add)
                    if t < T - 1:
                        for h_ in (0, 512):
                            nc.tensor.matmul(v2[:, h_:h_ + 512], negIT,
                                             st2a[:, t * HW + h_:
                                                  t * HW + h_ + 512],
                                             start=False, stop=False,
                                             skip_group_check=True)

                    OUT = psScr.tile([128, HW], F32, tag="scr")
                    sp3 = st['sps'][t]
                    for ti in range(9):
                        dy, dx = TAPS[ti]
                        cT = c128s('convT', q * 9 + ti)
                        rhs = sp3[:, 1 + dy:33 + dy, 1 + dx:33 + dx]
                        nc.tensor.matmul(OUT[:, 0:512], cT,
                                         rhs[:, 0:16, :],
                                         start=(ti == 0), stop=False,
                                         skip_group_check=True)
                        nc.tensor.matmul(OUT[:, 512:1024], cT,
                                         rhs[:, 16:32, :],
                                         start=(ti == 0), stop=False,
                                         skip_group_check=True)
                    for h_ in (0, 512):
                        nc.tensor.matmul(OUT[:, h_:h_ + 512], a2dTq,
                                         st2a[:, t * HW + h_:
                                              t * HW + h_ + 512],
                                         start=False, stop=False,
                                         skip_group_check=True)
                        nc.tensor.matmul(OUT[:, h_:h_ + 512],
                                         c128s('idI', t),
                                         st['xq'][:, t * HW + h_:
                                                  t * HW + h_ + 512],
                                         start=False, stop=True,
                                         skip_group_check=True)
                    osb = ostpool.tile([128, HW], F16, tag="ost")
                    nc.scalar.activation(osb[:, :], OUT[:, :],
                                         AF.Identity,
                                         bias=betaA[:, q:q + 1])
                    nc.sync.dma_start(
                        outd.ap()[q][:, t * HW:(t + 1) * HW], osb[:, :])

                def bfirst(t=0):
                    bpre()
                    bt(0)

                def z0(t=0):
                    zpre()
                    zrow(0)
                return ([bfirst] + [lambda t=t: bt(t) for t in (1, 2, 3)]
                        + [z0] + [lambda t=t: zrow(t) for t in (1, 2, 3)]
                        + [st3ch]
                        + [lambda t=t: wcol(t) for t in range(T)]
                        + [st4ch]
                        + [lambda t=t: dt(t) for t in range(T)])

            # ---- pipelined emission: H2(q) interleaved with H1(q+1) --
            fetch_x(0)
            sts = [dict() for _ in range(NQ)]
            emit_h1 = {q: h1_chunks(q, sts[q]) for q in range(NQ)}
            emit_h2 = {q: h2_chunks(q, sts[q]) for q in range(NQ)}

            for ch in emit_h1[0]:
                ch()
            for q in range(NQ):
                h2 = emit_h2[q]
                h1 = emit_h1[q + 1] if q + 1 < NQ else []
                i2, i1 = 0, 0
                while i2 < len(h2) or i1 < len(h1):
                    if i2 < len(h2):
                        h2[i2]()
                        i2 += 1
                    if i1 < len(h1) and i1 * len(h2) <= i2 * len(h1):
                        h1[i1]()
                        i1 += 1

    _split_excess_waits(nc)
    return nc


_NC_CACHE = None


def _get_nc():
    global _NC_CACHE
    if _NC_CACHE is None:
        _NC_CACHE = _build_program()
    return _NC_CACHE


def _prep_inputs(inputs):
    x = np.asarray(inputs['x'], np.float32)          # [T, B, C, H, W]
    c128, c1, betaAc = _host_consts(inputs)
    # x16[t] = 2^(t-1) * x_t: leak-free scaled LIF1 state form
    tscale = (2.0 ** (np.arange(T) - 1)).astype(np.float32)
    in_maps = []
    for b in range(NCORES):
        xb = (tscale[:, None, None, None] * x[:, b].reshape(T, C, Hh, Ww)) \
            .reshape(T, NQ, 128, HW).transpose(1, 2, 0, 3)
        m = {'x16h': np.ascontiguousarray(xb).reshape(NQ, 128, THW)
             .astype(np.float16),
             'c128': c128, 'c1': c1, 'betaA': betaAc}
        in_maps.append(m)
    return in_maps


def kernel(**inputs):
    in_maps = _prep_inputs(inputs)
    nc = _get_nc()
    res = run_bass_kernel_spmd(nc, in_maps, list(range(NCORES))).results
    # out [NQ, 128, T*HW] f16 -> [T, B, C, H, W] f32
    outs = []
    for b in range(NCORES):
        ob = res[b]['out'].astype(np.float32).reshape(NQ, 128, T, HW) \
            .transpose(2, 0, 1, 3)
        outs.append(ob.reshape(T, C, HW))
    out = np.stack(outs, axis=1)
    return out.reshape(T, Bb, C, Hh, Ww).astype(np.float32)


# revision 21
# speedup vs baseline: 1.1840x; 1.0520x over previous
"""FATM (wavelet spiking module) Trainium2 Bass kernel — v2.

Data-parallel over B across 8 NeuronCores (B=8 -> 1 sample/core).

v2 layout/op changes vs v1 (same validated layout algebra):
  - x shipped as f16 (0.5*x); u1 LIF state f16 -> LIF1 runs entirely in
    DVE 2x/4x perf modes (tensor_scalar/stt all-f16 SBUF unit-stride).
  - LIF1 hard reset via is_lt mask + multiply (no copy_predicated f32).
  - Stream transposes batched over all T=4 steps (one [128,4096] ST per
    stage instead of four [128,1024]).
  - negIF2 state lives in PSUM: mix/P0 matmuls accumulate straight into
    it; soft reset via -0.5*I matmul feedback (like negIF1). Kills the
    v2sb DVE add/reset and the MX evacuation.
  - bn0/LIF2 params materialized as full [128,1024] f16 tiles (DRAM
    consts) -> unit-stride operands, no stride-0 broadcast APs.
  - identity (+x) enters OUT PSUM via a 2*I matmul on f16 x; betaA bias
    applied by the ACT evacuation (bias AP); output DMA'd as f16.
  - Phased emission per q: LIF1(all t) -> ST1 -> colMM/evac(all t) ->
    ST2 -> per-t negIF1/LIF2 -> per-t invrow/evac -> ST3 -> invcol/evac
    -> ST4 -> per-t mix+negIF2 + conv/OUT/evac/DMA. PSUM: V1+v2 states
    (2x2 banks) + 2x[128,1024] scratch = 8 banks.
"""
import os
import sys
sys.path.insert(0, '/opt/trn_rl_repo')
sys.path.insert(0, '/root/.axon_site/_ro/trn_rl_repo')

import numpy as np

import bass_rust
from concourse import bass, mybir
import concourse.tile as tile_mod
from concourse.tile import TileContext
from concourse.vector_clock import ScopedClock
from concourse.bass_utils import run_bass_kernel_spmd

# ------------------------------------------------------------- walrus fix
MAX_WAITS = 1


def _patched_drain_and_barrier(self, tick_clock, wait_clock):
    drain_inst = self.nc.sync.drain()
    wait_clock.add_sem_waits(
        drain_inst.ins, ScopedClock({None: tick_clock.global_clock})
    )
    si = drain_inst.ins.sync_info
    if si is not None and si.on_wait and len(si.on_wait) > MAX_WAITS:
        waits = list(si.on_wait)
        si.on_wait = waits[:MAX_WAITS]
        for i in range(MAX_WAITS, len(waits), MAX_WAITS):
            nop = self.nc.sync.nop(nofuse=True, hint="wait_spill")
            nop.ins.sync_info = bass_rust.SyncInfo(
                on_wait=waits[i:i + MAX_WAITS], on_update=[]
            )
    self.nc.all_engine_barrier()
    assert self.sems is not None
    popped = self.nc._tile_sem_poison_stack.pop()
    assert popped is self._sem_poison
    self.nc.clear_and_free_semaphores(list(self.sems.allocated().values()))
    self.nc.all_engine_barrier()


tile_mod.TileContext._drain_and_barrier = _patched_drain_and_barrier


def _split_excess_waits(nc):
    """This walrus build rejects >1 sync wait per instruction; spill excess
    waits onto same-engine nops inserted before the instruction."""
    n_split = 0
    for bb in nc.main_func.blocks:
        insts = list(bb.instructions)
        out, changed = [], False
        for ins in insts:
            si = ins.sync_info
            if si is not None and si.on_wait and len(si.on_wait) > MAX_WAITS:
                waits = list(si.on_wait)
                si.on_wait = waits[-MAX_WAITS:]
                for i in range(0, len(waits) - MAX_WAITS, MAX_WAITS):
                    nop = mybir.InstNoOp(name=f"{ins.name}_wsp{i}", ins=[],
                                         outs=[])
                    nop.engine = ins.engine
                    nop.sync_info = bass_rust.SyncInfo(
                        on_wait=waits[i:i + MAX_WAITS], on_update=[])
                    out.append(nop)
                    n_split += 1
                changed = True
            out.append(ins)
        if changed:
            try:
                bb.instructions = out
            except Exception:
                lst = bb.instructions
                lst.clear()
                lst.extend(out)
    return n_split


# ---------------------------------------------------------------- consts
EPS = 1e-5
T, Bb, C, Hh, Ww = 4, 8, 512, 32, 32
NQ, HW = 4, 1024
THW = T * HW
NCORES = 8
F32 = mybir.dt.float32
F16 = mybir.dt.float16
U16 = mybir.dt.uint16
ALU = mybir.AluOpType
AF = mybir.ActivationFunctionType
TAPS = [(dy, dx) for dy in (-1, 0, 1) for dx in (-1, 0, 1)]

# f16 [128, x] consts packed into one DRAM array (order defines offsets)
PACK128 = [
    ('lfwdT', 128), ('linvT', 128), ('negIT', 128), ('idI', T * 128),
    ('mixT', NQ * 128), ('convT', NQ * 9 * 128), ('a2dT', NQ * 128),
    ('inv0cF', NQ * HW), ('thr2F', NQ * HW), ('negBF', NQ * HW),
]
PACK1 = [('p0row', HW), ('beta1', NQ * 128)]


def _haar_matrix(n):
    h = np.array([[1.0]])
    while h.shape[0] < n:
        top = np.kron(h, [1.0, 1.0])
        bot = np.kron(np.eye(h.shape[0]), [1.0, -1.0])
        h = np.concatenate([top, bot], axis=0) / np.sqrt(2.0)
    return h


def _bd4(block_fn):
    L = np.zeros((128, 128))
    for cb in range(4):
        L[32 * cb:32 * cb + 32, 32 * cb:32 * cb + 32] = block_fn(cb)
    return L


def _host_consts(inputs):
    hw_ = np.asarray(inputs['haar_weight'], np.float64)
    w1 = np.asarray(inputs['conv1_w'], np.float64)[:, :, 0, 0]
    b1 = np.asarray(inputs['conv1_b'], np.float64)
    w2 = np.asarray(inputs['conv2_w'], np.float64)
    b2 = np.asarray(inputs['conv2_b'], np.float64)
    bnw = np.asarray(inputs['bn_weight'], np.float64)
    bnb = np.asarray(inputs['bn_bias'], np.float64)
    bnm = np.asarray(inputs['bn_mean'], np.float64)
    bnv = np.asarray(inputs['bn_var'], np.float64)
    inv = bnw / np.sqrt(bnv + EPS)
    bbias = bnb - bnm * inv

    Q = _haar_matrix(32)
    P0flat = (Q.T @ np.ones((32, 32)) @ Q).reshape(HW)

    d = {}
    d['lfwdT'] = _bd4(lambda cb: Q.T)        # [p=(cb,w), m=(cb,l)] = Q[l,w]
    d['linvT'] = _bd4(lambda cb: Q)          # [p=(cb,i), m=(cb,a)] = Q[i,a]
    d['negIT'] = -0.5 * np.eye(128)
    # identity-add weights per t: x16[t] = 2^(t-1)*x_t, so x_t = 2^(1-t)*x16
    d['idI'] = np.concatenate(
        [2.0 ** (1 - t) * np.eye(128) for t in range(T)], axis=1)
    d['p0row'] = P0flat.reshape(1, HW)

    mixT = np.zeros((NQ, 128, 128))
    convT = np.zeros((NQ, 9, 128, 128))
    a2dT = np.zeros((NQ, 128, 128))
    beta1 = np.zeros((NQ, 128))
    betaA = np.zeros((NQ, 128))
    inv0cF = np.zeros((NQ, 128, HW))
    thr2F = np.zeros((NQ, 128, HW))
    negBF = np.zeros((NQ, 128, HW))

    for q in range(NQ):
        cidx = np.arange(128 * q, 128 * q + 128)
        inv0q, bias0q = inv[0][cidx], bbias[0][cidx]
        inv1q, bias1q = inv[1][cidx], bbias[1][cidx]
        inv2q, bias2q = inv[2][cidx], bbias[2][cidx]
        inv3q, bias3q = inv[3][cidx], bbias[3][cidx]
        inv4q, bias4q = inv[4][cidx], bbias[4][cidx]

        mixT[q] = _bd4(
            lambda cb: hw_[4 * q + cb] * inv1q.reshape(4, 32)[cb][None, :])
        for ti, (dy, dx) in enumerate(TAPS):
            def cb_blk(cb, dy=dy, dx=dx):
                m = w2[:, :, dy + 1, dx + 1].T * \
                    inv4q.reshape(4, 32)[cb][None, :]
                if dy == 0 and dx == 0:
                    m = m + w1.T * inv3q.reshape(4, 32)[cb][None, :]
                return m
            convT[q, ti] = _bd4(cb_blk)
        a2dT[q] = np.diag(inv2q / 2.0)
        beta1[q] = bias1q
        betaA[q] = (inv4q * np.tile(b2, 16)[cidx] + bias4q
                    + inv3q * np.tile(b1, 16)[cidx] + bias3q + bias2q)

        # full [128, (cc,l)] param tiles in the transformed layout:
        # value at (p=(cb,i), f=(cc*32+l)) = param[q, cb, cc]
        def full_tile(v):          # v: [128] per-channel (cb,cc)
            blk = v.reshape(4, 32)                      # [cb, cc]
            t_ = np.repeat(blk[:, None, :], 32, axis=1)  # [cb, i, cc]
            t_ = t_.reshape(128, 32)                    # [p, cc]
            return np.repeat(t_[:, :, None], 32, axis=2).reshape(128, HW)

        inv0cF[q] = full_tile(inv0q / 4.0)
        thr2F[q] = full_tile(1.0 - bias0q)
        negBF[q] = full_tile(-bias0q)

    d['mixT'] = mixT.transpose(1, 0, 2).reshape(128, NQ * 128)
    d['convT'] = convT.transpose(2, 0, 1, 3).reshape(128, NQ * 9 * 128)
    d['a2dT'] = a2dT.transpose(1, 0, 2).reshape(128, NQ * 128)
    d['inv0cF'] = inv0cF.transpose(1, 0, 2).reshape(128, NQ * HW)
    d['thr2F'] = thr2F.transpose(1, 0, 2).reshape(128, NQ * HW)
    d['negBF'] = negBF.transpose(1, 0, 2).reshape(128, NQ * HW)
    d['beta1'] = beta1.reshape(1, NQ * 128)

    c128 = np.concatenate(
        [np.asarray(d[n]).reshape(128, w) for n, w in PACK128],
        axis=1).astype(np.float16)
    c1 = np.concatenate(
        [np.asarray(d[n]).reshape(1, w) for n, w in PACK1],
        axis=1).astype(np.float16)
    betaAc = np.ascontiguousarray(betaA.T).astype(np.float32)  # [128, NQ]
    return np.ascontiguousarray(c128), np.ascontiguousarray(c1), betaAc


P128_W = sum(w for _, w in PACK128)
P1_W = sum(w for _, w in PACK1)
P128_OFF = {}
_o = 0
for _n, _w in PACK128:
    P128_OFF[_n] = (_o, _w)
    _o += _w
P1_OFF = {}
_o = 0
for _n, _w in PACK1:
    P1_OFF[_n] = (_o, _w)
    _o += _w


def _build_program():
    nc = bass.Bass("TRN2", target_bir_lowering=False, debug=False)
    x16d = nc.declare_dram_parameter("x16h", [NQ, 128, THW], F16,
                                     isOutput=False)
    c128d = nc.declare_dram_parameter("c128", [128, P128_W], F16,
                                      isOutput=False)
    c1d = nc.declare_dram_parameter("c1", [1, P1_W], F16, isOutput=False)
    betaAd = nc.declare_dram_parameter("betaA", [128, NQ], F32,
                                       isOutput=False)
    outd = nc.declare_dram_parameter("out", [NQ, 128, THW], F16,
                                     isOutput=True)

    with TileContext(nc) as tc:
        with (
            tc.tile_pool(name="consts", bufs=1) as cpool,
            tc.tile_pool(name="xq", bufs=3) as xqpool,
            tc.tile_pool(name="big", bufs=1) as bigpool,
            tc.tile_pool(name="state", bufs=1) as spool,
            tc.tile_pool(name="spp", bufs=7) as sppool,
            tc.tile_pool(name="wk", bufs=2) as wpool,
            tc.tile_pool(name="ost", bufs=2) as ostpool,
            tc.tile_pool(name="psSt", bufs=1, space="PSUM") as psSt,
            tc.tile_pool(name="psScr", bufs=2, space="PSUM") as psScr,
        ):
            c128 = cpool.tile([128, P128_W], F16, tag="c128")
            nc.sync.dma_start(c128[:, :], c128d.ap())
            c1 = cpool.tile([1, P1_W], F16, tag="c1")
            nc.sync.dma_start(c1[:, :], c1d.ap())
            betaA = cpool.tile([128, NQ], F32, tag="betaA")
            nc.sync.dma_start(betaA[:, :], betaAd.ap())
            one_m1 = cpool.tile([128, 1], F32, tag="bm1")
            one_p1 = cpool.tile([128, 1], F32, tag="bp1")
            nc.vector.memset(one_m1[:, :], -1.0)
            nc.vector.memset(one_p1[:, :], 1.0)

            def c128s(name, idx=0, w=128):
                off, _ = P128_OFF[name]
                return c128[:, off + idx * w: off + (idx + 1) * w]

            def c1s(name, idx=0, w=None):
                off, tot = P1_OFF[name]
                if w is None:
                    w = tot
                return c1[0:1, off + idx * w: off + (idx + 1) * w]

            lfwdT = c128s('lfwdT')
            linvT = c128s('linvT')
            negIT = c128s('negIT')

            xqs = {}

            def fetch_x(q):
                xq = xqpool.tile([128, THW], F16, tag="xq")
                nc.sync.dma_start(xq[:, :], x16d.ap()[q])
                xqs[q] = xq

            # ---- per-q chunk builders; st holds per-q tiles ----
            def h1_chunks(q, st):
                """front half: LIF1 (scaled, leak-free), ST1, colMMs, ST2"""
                def pre():
                    fetch_x(q + 1) if q + 1 < NQ else None
                    st['xq'] = xqs[q]
                    st['u1h'] = spool.tile([128, HW], F16, tag="u1h", name=f"u1h{q}")
                    nc.vector.memset(st['u1h'][:, :], 0.0)
                    st['sN'] = bigpool.tile([128, THW], F16, tag="sN", name=f"sN{q}")
                    st['sps'] = []

                def lif1(t):
                    u1h = st['u1h']
                    thr = float(2.0 ** t)
                    nc.vector.tensor_tensor(
                        u1h[:, :], u1h[:, :],
                        st['xq'][:, t * HW:(t + 1) * HW], ALU.add)
                    sp = sppool.tile([128, 34 * 34], F16, tag="sp")
                    if q == 0 or (q == 1 and t < 3):
                        # 7 pool bufs; interiors always overwritten,
                        # borders must be zero once per buf
                        nc.gpsimd.memset(sp[:, :], 0.0)
                    sp3 = sp[:, :].rearrange("p (h w) -> p h w", h=34,
                                             w=34)
                    nc.vector.tensor_scalar(
                        sp3[:, 1:33, 1:33],
                        u1h[:, :].rearrange("p (h w) -> p h w", h=32,
                                            w=32),
                        thr, None, ALU.is_ge)
                    nc.vector.tensor_scalar(
                        st['sN'][:, t * HW:(t + 1) * HW], u1h[:, :], thr,
                        None, ALU.is_ge)
                    ns = wpool.tile([128, HW], F16, tag="ns")
                    nc.vector.tensor_scalar(ns[:, :], u1h[:, :], thr,
                                            None, ALU.is_lt)
                    if t < T - 1:
                        nc.vector.tensor_tensor(u1h[:, :], u1h[:, :],
                                                ns[:, :], ALU.mult)
                    st['sps'].append(sp3)

                def st1ch():
                    st['sD'] = bigpool.tile([128, THW], F16, tag="sD", name=f"sD{q}")
                    nc.vector.transpose(st['sD'][:, :], st['sN'][:, :])
                    st['M1s'] = bigpool.tile([128, THW], F16, tag="M1s", name=f"M1s{q}")

                def colmm(t):
                    M1 = psScr.tile([128, HW], F32, tag="scr")
                    sD3 = st['sD'][:, t * HW:(t + 1) * HW].rearrange(
                        "p (h cc) -> p h cc", h=32, cc=32)
                    for j in (0, 1):
                        nc.tensor.matmul(
                            M1[:, 512 * j:512 * j + 512].rearrange(
                                "p (cc h) -> p h cc", cc=16, h=32),
                            lfwdT, sD3[:, :, 16 * j:16 * j + 16],
                            start=True, stop=True)
                    nc.scalar.copy(st['M1s'][:, t * HW:(t + 1) * HW],
                                   M1[:, :])

                def st2ch():
                    st['sB'] = bigpool.tile([128, THW], F16, tag="sB", name=f"sB{q}")
                    nc.vector.transpose(st['sB'][:, :], st['M1s'][:, :])

                def first(t=0):
                    pre()
                    lif1(0)
                return ([first] + [lambda t=t: lif1(t) for t in (1, 2, 3)]
                        + [st1ch]
                        + [lambda t=t: colmm(t) for t in range(T)]
                        + [st2ch])

            def h2_chunks(q, st):
                """back half: negIF1+LIF2, inverse haar, mix+negIF2+out"""
                inv0cF = c128s('inv0cF', q, HW)
                thr2F = c128s('thr2F', q, HW)
                negBF = c128s('negBF', q, HW)
                mixTq = c128s('mixT', q)
                a2dTq = c128s('a2dT', q)

                def bpre():
                    st['V1'] = psSt.tile([128, HW], F32, tag="V1", name=f"V1{q}")
                    st['u2h'] = spool.tile([128, HW], F16, tag="u2h", name=f"u2h{q}")
                    nc.vector.tensor_copy(st['u2h'][:, :], negBF)
                    st['s2a'] = bigpool.tile([128, THW], F16, tag="s2a", name=f"s2a{q}")

                def bt(t):
                    V1 = st['V1']
                    u2h = st['u2h']
                    for h_ in (0, 512):
                        nc.tensor.matmul(
                            V1[:, h_:h_ + 512], lfwdT,
                            st['sB'][:, t * HW + h_:t * HW + h_ + 512],
                            start=(t == 0), stop=(t == T - 1),
                            skip_group_check=True)
                    g1 = wpool.tile([128, HW], F16, tag="g1")
                    g2 = wpool.tile([128, HW], F16, tag="g2")
                    nc.scalar.activation(g1[:, :], V1[:, :], AF.Sign,
                                         bias=one_m1[:, :])
                    nc.scalar.activation(g2[:, :], V1[:, :], AF.Sign,
                                         bias=one_p1[:, :])
                    st1 = wpool.tile([128, HW], F16, tag="st1")
                    nc.gpsimd.tensor_tensor(st1[:, :], g1[:, :], g2[:, :],
                                            ALU.add)
                    if t < T - 1:
                        for h_ in (0, 512):
                            nc.tensor.matmul(V1[:, h_:h_ + 512], negIT,
                                             st1[:, h_:h_ + 512],
                                             start=False, stop=False,
                                             skip_group_check=True)
                    pprod = wpool.tile([128, HW], F16, tag="pprod")
                    nc.vector.tensor_tensor(pprod[:, :], st1[:, :],
                                            inv0cF, ALU.mult)
                    nc.vector.tensor_scalar(u2h[:, :], u2h[:, :], 0.5,
                                            None, ALU.mult)
                    nc.vector.tensor_tensor(u2h[:, :], u2h[:, :],
                                            pprod[:, :], ALU.add)
                    s2t = st['s2a'][:, t * HW:(t + 1) * HW]
                    nc.vector.tensor_tensor(s2t, u2h[:, :], thr2F,
                                            ALU.is_ge)
                    nc.vector.copy_predicated(u2h[:, :],
                                              s2t.bitcast(U16), negBF)

                def zpre():
                    st['Zs'] = bigpool.tile([128, THW], F16, tag="Zs", name=f"Zs{q}")

                def zrow(t):
                    Z = psScr.tile([128, HW], F32, tag="scr")
                    for h_ in (0, 512):
                        nc.tensor.matmul(
                            Z[:, h_:h_ + 512], linvT,
                            st['s2a'][:, t * HW + h_:t * HW + h_ + 512],
                            start=True, stop=True)
                    nc.scalar.copy(st['Zs'][:, t * HW:(t + 1) * HW],
                                   Z[:, :])

                def st3ch():
                    st['ZT'] = bigpool.tile([128, THW], F16, tag="ZT", name=f"ZT{q}")
                    nc.vector.transpose(st['ZT'][:, :], st['Zs'][:, :])
                    st['Ws'] = bigpool.tile([128, THW], F16, tag="Ws", name=f"Ws{q}")

                def wcol(t):
                    W2 = psScr.tile([128, HW], F32, tag="scr")
                    ZT3 = st['ZT'][:, t * HW:(t + 1) * HW].rearrange(
                        "p (cc a) -> p cc a", cc=32, a=32)
                    for j in (0, 1):
                        nc.tensor.matmul(
                            W2[:, 512 * j:512 * j + 512].rearrange(
                                "p (a cc) -> p cc a", a=16, cc=32),
                            linvT, ZT3[:, :, 16 * j:16 * j + 16],
                            start=True, stop=True)
                    nc.scalar.copy(st['Ws'][:, t * HW:(t + 1) * HW],
                                   W2[:, :])

                def st4ch():
                    st['haarA'] = bigpool.tile([128, THW], F16, tag="haarA", name=f"haarA{q}")
                    nc.vector.transpose(st['haarA'][:, :], st['Ws'][:, :])
                    st['v2'] = psSt.tile([128, HW], F32, tag="v2", name=f"v2{q}")
                    st['st2a'] = bigpool.tile([128, THW], F16, tag="st2a", name=f"st2a{q}")

                def dt(t):
                    v2 = st['v2']
                    st2a = st['st2a']
                    for h_ in (0, 512):
                        nc.tensor.matmul(v2[:, h_:h_ + 512], mixTq,
                                         st['haarA'][:, t * HW + h_:
                                                     t * HW + h_ + 512],
                                         start=(t == 0), stop=False,
                                         skip_group_check=True)
                        nc.tensor.matmul(v2[:, h_:h_ + 512],
                                         c1s('beta1', q, 128),
                                         c1s('p0row')[0:1, h_:h_ + 512],
                                         start=False, stop=(t == T - 1),
                                         skip_group_check=True)
                    g1b = wpool.tile([128, HW], F16, tag="g1b")
                    g2b = wpool.tile([128, HW], F16, tag="g2b")
                    nc.scalar.activation(g1b[:, :], v2[:, :], AF.Sign,
                                         bias=one_m1[:, :])
                    nc.scalar.activation(g2b[:, :], v2[:, :], AF.Sign,
                                         bias=one_p1[:, :])
                    st2t = st2a[:, t * HW:(t + 1) * HW]
                    nc.gpsimd.tensor_tensor(st2t, g1b[:, :], g2b[:, :],
                                            ALU.add)
                    if t < T - 1:
                        for h_ in (0, 512):
                            nc.tensor.matmul(v2[:, h_:h_ + 512], negIT,
                                             st2a[:, t * HW + h_:
                                                  t * HW + h_ + 512],
                                             start=False, stop=False,
                                             skip_group_check=True)

                    OUT = psScr.tile([128, HW], F32, tag="scr")
                    sp3 = st['sps'][t]
                    for ti in range(9):
                        dy, dx = TAPS[ti]
                        cT = c128s('convT', q * 9 + ti)
                        rhs = sp3[:, 1 + dy:33 + dy, 1 + dx:33 + dx]
                        nc.tensor.matmul(OUT[:, 0:512], cT,
                                         rhs[:, 0:16, :],
                                         start=(ti == 0), stop=False,
                                         skip_group_check=True)
                        nc.tensor.matmul(OUT[:, 512:1024], cT,
                                         rhs[:, 16:32, :],
                                         start=(ti == 0), stop=False,
                                         skip_group_check=True)
                    for h_ in (0, 512):
                        nc.tensor.matmul(OUT[:, h_:h_ + 512], a2dTq,
                                         st2a[:, t * HW + h_:
                                              t * HW + h_ + 512],
                                         start=False, stop=False,
                                         skip_group_check=True)
                        nc.tensor.matmul(OUT[:, h_:h_ + 512],
                                         c128s('idI', t),
                                         st['xq'][:, t * HW + h_:
                                                  t * HW + h_ + 512],
                                         start=False, stop=True,
                                         skip_group_check=True)
                    osb = ostpool.tile([128, HW], F16, tag="ost")
                    nc.scalar.activation(osb[:, :], OUT[:, :],
                                         AF.Identity,
                                         bias=betaA[:, q:q + 1])
                    nc.sync.dma_start(
                        outd.ap()[q][:, t * HW:(t + 1) * HW], osb[:, :])

                def bfirst(t=0):
                    bpre()
                    bt(0)

                def z0(t=0):
                    zpre()
                    zrow(0)
                return ([bfirst] + [lambda t=t: bt(t) for t in (1, 2, 3)]
                        + [z0] + [lambda t=t: zrow(t) for t in (1, 2, 3)]
                        + [st3ch]
                        + [lambda t=t: wcol(t) for t in range(T)]
                        + [st4ch]
                        + [lambda t=t: dt(t) for t in range(T)])

            # ---- rolling pipelined emission across q ----------------
            # q's chunk list L(q) has 27 entries; q+1 starts OFF chunks
            # after q. Smaller OFF = deeper overlap; floor ~14 set by
            # xq triple-buffering.
            OFF = int(os.environ.get('FATM_OFF', '16'))
            fetch_x(0)
            sts = [dict() for _ in range(NQ)]
            Ls = [h1_chunks(q, sts[q]) + h2_chunks(q, sts[q])
                  for q in range(NQ)]
            nch = len(Ls[0])
            for s in range((NQ - 1) * OFF + nch):
                for q in range(NQ):
                    c = s - q * OFF
                    if 0 <= c < nch:
                        Ls[q][c]()

    _split_excess_waits(nc)
    return nc


_NC_CACHE = None


def _get_nc():
    global _NC_CACHE
    if _NC_CACHE is None:
        _NC_CACHE = _build_program()
    return _NC_CACHE


def _prep_inputs(inputs):
    x = np.asarray(inputs['x'], np.float32)          # [T, B, C, H, W]
    c128, c1, betaAc = _host_consts(inputs)
    # x16[t] = 2^(t-1) * x_t: leak-free scaled LIF1 state form
    tscale = (2.0 ** (np.arange(T) - 1)).astype(np.float32)
    in_maps = []
    for b in range(NCORES):
        xb = (tscale[:, None, None, None] * x[:, b].reshape(T, C, Hh, Ww)) \
            .reshape(T, NQ, 128, HW).transpose(1, 2, 0, 3)
        m = {'x16h': np.ascontiguousarray(xb).reshape(NQ, 128, THW)
             .astype(np.float16),
             'c128': c128, 'c1': c1, 'betaA': betaAc}
        in_maps.append(m)
    return in_maps


def kernel(**inputs):
    in_maps = _prep_inputs(inputs)
    nc = _get_nc()
    res = run_bass_kernel_spmd(nc, in_maps, list(range(NCORES))).results
    # out [NQ, 128, T*HW] f16 -> [T, B, C, H, W] f32
    outs = []
    for b in range(NCORES):
        ob = res[b]['out'].astype(np.float32).reshape(NQ, 128, T, HW) \
            .transpose(2, 0, 1, 3)
        outs.append(ob.reshape(T, C, HW))
    out = np.stack(outs, axis=1)
    return out.reshape(T, Bb, C, Hh, Ww).astype(np.float32)


# revision 27
# speedup vs baseline: 1.2121x; 1.0238x over previous
"""FATM (wavelet spiking module) Trainium2 Bass kernel — v2.

Data-parallel over B across 8 NeuronCores (B=8 -> 1 sample/core).

v2 layout/op changes vs v1 (same validated layout algebra):
  - x shipped as f16 (0.5*x); u1 LIF state f16 -> LIF1 runs entirely in
    DVE 2x/4x perf modes (tensor_scalar/stt all-f16 SBUF unit-stride).
  - LIF1 hard reset via is_lt mask + multiply (no copy_predicated f32).
  - Stream transposes batched over all T=4 steps (one [128,4096] ST per
    stage instead of four [128,1024]).
  - negIF2 state lives in PSUM: mix/P0 matmuls accumulate straight into
    it; soft reset via -0.5*I matmul feedback (like negIF1). Kills the
    v2sb DVE add/reset and the MX evacuation.
  - bn0/LIF2 params materialized as full [128,1024] f16 tiles (DRAM
    consts) -> unit-stride operands, no stride-0 broadcast APs.
  - identity (+x) enters OUT PSUM via a 2*I matmul on f16 x; betaA bias
    applied by the ACT evacuation (bias AP); output DMA'd as f16.
  - Phased emission per q: LIF1(all t) -> ST1 -> colMM/evac(all t) ->
    ST2 -> per-t negIF1/LIF2 -> per-t invrow/evac -> ST3 -> invcol/evac
    -> ST4 -> per-t mix+negIF2 + conv/OUT/evac/DMA. PSUM: V1+v2 states
    (2x2 banks) + 2x[128,1024] scratch = 8 banks.
"""
import os
import sys
sys.path.insert(0, '/opt/trn_rl_repo')
sys.path.insert(0, '/root/.axon_site/_ro/trn_rl_repo')

import numpy as np

import bass_rust
from concourse import bass, mybir
import concourse.tile as tile_mod
from concourse.tile import TileContext
from concourse.vector_clock import ScopedClock
from concourse.bass_utils import run_bass_kernel_spmd

# ------------------------------------------------------------- walrus fix
MAX_WAITS = 1


def _patched_drain_and_barrier(self, tick_clock, wait_clock):
    drain_inst = self.nc.sync.drain()
    wait_clock.add_sem_waits(
        drain_inst.ins, ScopedClock({None: tick_clock.global_clock})
    )
    si = drain_inst.ins.sync_info
    if si is not None and si.on_wait and len(si.on_wait) > MAX_WAITS:
        waits = list(si.on_wait)
        si.on_wait = waits[:MAX_WAITS]
        for i in range(MAX_WAITS, len(waits), MAX_WAITS):
            nop = self.nc.sync.nop(nofuse=True, hint="wait_spill")
            nop.ins.sync_info = bass_rust.SyncInfo(
                on_wait=waits[i:i + MAX_WAITS], on_update=[]
            )
    self.nc.all_engine_barrier()
    assert self.sems is not None
    popped = self.nc._tile_sem_poison_stack.pop()
    assert popped is self._sem_poison
    self.nc.clear_and_free_semaphores(list(self.sems.allocated().values()))
    self.nc.all_engine_barrier()


tile_mod.TileContext._drain_and_barrier = _patched_drain_and_barrier


def _split_excess_waits(nc):
    """This walrus build rejects >1 sync wait per instruction; spill excess
    waits onto same-engine nops inserted before the instruction."""
    n_split = 0
    for bb in nc.main_func.blocks:
        insts = list(bb.instructions)
        out, changed = [], False
        for ins in insts:
            si = ins.sync_info
            if si is not None and si.on_wait and len(si.on_wait) > MAX_WAITS:
                waits = list(si.on_wait)
                si.on_wait = waits[-MAX_WAITS:]
                for i in range(0, len(waits) - MAX_WAITS, MAX_WAITS):
                    nop = mybir.InstNoOp(name=f"{ins.name}_wsp{i}", ins=[],
                                         outs=[])
                    nop.engine = ins.engine
                    nop.sync_info = bass_rust.SyncInfo(
                        on_wait=waits[i:i + MAX_WAITS], on_update=[])
                    out.append(nop)
                    n_split += 1
                changed = True
            out.append(ins)
        if changed:
            try:
                bb.instructions = out
            except Exception:
                lst = bb.instructions
                lst.clear()
                lst.extend(out)
    return n_split


# ---------------------------------------------------------------- consts
EPS = 1e-5
T, Bb, C, Hh, Ww = 4, 8, 512, 32, 32
NQ, HW = 4, 1024
THW = T * HW
NCORES = 8
F32 = mybir.dt.float32
F16 = mybir.dt.float16
F8 = mybir.dt.float8e4
U16 = mybir.dt.uint16
ALU = mybir.AluOpType
AF = mybir.ActivationFunctionType
TAPS = [(dy, dx) for dy in (-1, 0, 1) for dx in (-1, 0, 1)]

# f16 [128, x] consts packed into one DRAM array (order defines offsets)
PACK128 = [
    ('lfwdT', 128), ('linvT', 128), ('negIT', 128), ('idI', T * 128),
    ('mixT', NQ * 128), ('convT', NQ * 9 * 128), ('a2dT', NQ * 128),
    ('inv0cF', NQ * HW), ('thr2F', NQ * HW), ('negBF', NQ * HW),
]
PACK1 = [('p0row', HW), ('beta1', NQ * 128)]


def _haar_matrix(n):
    h = np.array([[1.0]])
    while h.shape[0] < n:
        top = np.kron(h, [1.0, 1.0])
        bot = np.kron(np.eye(h.shape[0]), [1.0, -1.0])
        h = np.concatenate([top, bot], axis=0) / np.sqrt(2.0)
    return h


def _bd4(block_fn):
    L = np.zeros((128, 128))
    for cb in range(4):
        L[32 * cb:32 * cb + 32, 32 * cb:32 * cb + 32] = block_fn(cb)
    return L


def _host_consts(inputs):
    hw_ = np.asarray(inputs['haar_weight'], np.float64)
    w1 = np.asarray(inputs['conv1_w'], np.float64)[:, :, 0, 0]
    b1 = np.asarray(inputs['conv1_b'], np.float64)
    w2 = np.asarray(inputs['conv2_w'], np.float64)
    b2 = np.asarray(inputs['conv2_b'], np.float64)
    bnw = np.asarray(inputs['bn_weight'], np.float64)
    bnb = np.asarray(inputs['bn_bias'], np.float64)
    bnm = np.asarray(inputs['bn_mean'], np.float64)
    bnv = np.asarray(inputs['bn_var'], np.float64)
    inv = bnw / np.sqrt(bnv + EPS)
    bbias = bnb - bnm * inv

    Q = _haar_matrix(32)
    P0flat = (Q.T @ np.ones((32, 32)) @ Q).reshape(HW)

    d = {}
    d['lfwdT'] = _bd4(lambda cb: Q.T)        # [p=(cb,w), m=(cb,l)] = Q[l,w]
    d['linvT'] = _bd4(lambda cb: Q)          # [p=(cb,i), m=(cb,a)] = Q[i,a]
    d['negIT'] = -0.5 * np.eye(128)
    # identity-add weights per t: x16[t] = 2^(t-1)*x_t, so x_t = 2^(1-t)*x16
    d['idI'] = np.concatenate(
        [2.0 ** (1 - t) * np.eye(128) for t in range(T)], axis=1)
    d['p0row'] = P0flat.reshape(1, HW)

    mixT = np.zeros((NQ, 128, 128))
    convT = np.zeros((NQ, 9, 128, 128))
    a2dT = np.zeros((NQ, 128, 128))
    beta1 = np.zeros((NQ, 128))
    betaA = np.zeros((NQ, 128))
    inv0cF = np.zeros((NQ, 128, HW))
    thr2F = np.zeros((NQ, 128, HW))
    negBF = np.zeros((NQ, 128, HW))

    for q in range(NQ):
        cidx = np.arange(128 * q, 128 * q + 128)
        inv0q, bias0q = inv[0][cidx], bbias[0][cidx]
        inv1q, bias1q = inv[1][cidx], bbias[1][cidx]
        inv2q, bias2q = inv[2][cidx], bbias[2][cidx]
        inv3q, bias3q = inv[3][cidx], bbias[3][cidx]
        inv4q, bias4q = inv[4][cidx], bbias[4][cidx]

        mixT[q] = _bd4(
            lambda cb: hw_[4 * q + cb] * inv1q.reshape(4, 32)[cb][None, :])
        for ti, (dy, dx) in enumerate(TAPS):
            def cb_blk(cb, dy=dy, dx=dx):
                m = w2[:, :, dy + 1, dx + 1].T * \
                    inv4q.reshape(4, 32)[cb][None, :]
                if dy == 0 and dx == 0:
                    m = m + w1.T * inv3q.reshape(4, 32)[cb][None, :]
                return m
            convT[q, ti] = _bd4(cb_blk)
        a2dT[q] = np.diag(inv2q / 2.0)
        beta1[q] = bias1q
        betaA[q] = (inv4q * np.tile(b2, 16)[cidx] + bias4q
                    + inv3q * np.tile(b1, 16)[cidx] + bias3q + bias2q)

        # full [128, (cc,l)] param tiles in the transformed layout:
        # value at (p=(cb,i), f=(cc*32+l)) = param[q, cb, cc]
        def full_tile(v):          # v: [128] per-channel (cb,cc)
            blk = v.reshape(4, 32)                      # [cb, cc]
            t_ = np.repeat(blk[:, None, :], 32, axis=1)  # [cb, i, cc]
            t_ = t_.reshape(128, 32)                    # [p, cc]
            return np.repeat(t_[:, :, None], 32, axis=2).reshape(128, HW)

        inv0cF[q] = full_tile(inv0q / 4.0)
        thr2F[q] = full_tile(1.0 - bias0q)
        negBF[q] = full_tile(-bias0q)

    d['mixT'] = mixT.transpose(1, 0, 2).reshape(128, NQ * 128)
    d['convT'] = convT.transpose(2, 0, 1, 3).reshape(128, NQ * 9 * 128)
    d['a2dT'] = a2dT.transpose(1, 0, 2).reshape(128, NQ * 128)
    d['inv0cF'] = inv0cF.transpose(1, 0, 2).reshape(128, NQ * HW)
    d['thr2F'] = thr2F.transpose(1, 0, 2).reshape(128, NQ * HW)
    d['negBF'] = negBF.transpose(1, 0, 2).reshape(128, NQ * HW)
    d['beta1'] = beta1.reshape(1, NQ * 128)

    c128 = np.concatenate(
        [np.asarray(d[n]).reshape(128, w) for n, w in PACK128],
        axis=1).astype(np.float16)
    c1 = np.concatenate(
        [np.asarray(d[n]).reshape(1, w) for n, w in PACK1],
        axis=1).astype(np.float16)
    betaAc = np.ascontiguousarray(betaA.T).astype(np.float32)  # [128, NQ]
    return np.ascontiguousarray(c128), np.ascontiguousarray(c1), betaAc


P128_W = sum(w for _, w in PACK128)
P1_W = sum(w for _, w in PACK1)
P128_OFF = {}
_o = 0
for _n, _w in PACK128:
    P128_OFF[_n] = (_o, _w)
    _o += _w
P1_OFF = {}
_o = 0
for _n, _w in PACK1:
    P1_OFF[_n] = (_o, _w)
    _o += _w


def _build_program():
    nc = bass.Bass("TRN2", target_bir_lowering=False, debug=False)
    x16d = nc.declare_dram_parameter("x16h", [NQ, 128, THW], F16,
                                     isOutput=False)
    c128d = nc.declare_dram_parameter("c128", [128, P128_W], F16,
                                      isOutput=False)
    c1d = nc.declare_dram_parameter("c1", [1, P1_W], F16, isOutput=False)
    betaAd = nc.declare_dram_parameter("betaA", [128, NQ], F32,
                                       isOutput=False)
    outd = nc.declare_dram_parameter("out", [NQ, 128, THW], F16,
                                     isOutput=True)

    with TileContext(nc) as tc:
        with (
            tc.tile_pool(name="consts", bufs=1) as cpool,
            tc.tile_pool(name="xq", bufs=2) as xqpool,
            tc.tile_pool(name="xqd", bufs=4) as xqdpool,
            tc.tile_pool(name="big", bufs=1) as bigpool,
            tc.tile_pool(name="state", bufs=1) as spool,
            tc.tile_pool(name="spp", bufs=12) as sppool,
            tc.tile_pool(name="wk", bufs=2) as wpool,
            tc.tile_pool(name="ost", bufs=2) as ostpool,
            tc.tile_pool(name="psSt", bufs=1, space="PSUM") as psSt,
            tc.tile_pool(name="psScr", bufs=2, space="PSUM") as psScr,
        ):
            c128 = cpool.tile([128, P128_W], F16, tag="c128")
            nc.sync.dma_start(c128[:, :], c128d.ap())
            c1 = cpool.tile([1, P1_W], F16, tag="c1")
            nc.sync.dma_start(c1[:, :], c1d.ap())
            betaA = cpool.tile([128, NQ], F32, tag="betaA")
            nc.sync.dma_start(betaA[:, :], betaAd.ap())
            one_m1 = cpool.tile([128, 1], F32, tag="bm1")
            one_p1 = cpool.tile([128, 1], F32, tag="bp1")
            nc.vector.memset(one_m1[:, :], -1.0)
            nc.vector.memset(one_p1[:, :], 1.0)

            def c128s(name, idx=0, w=128):
                off, _ = P128_OFF[name]
                return c128[:, off + idx * w: off + (idx + 1) * w]

            def c1s(name, idx=0, w=None):
                off, tot = P1_OFF[name]
                if w is None:
                    w = tot
                return c1[0:1, off + idx * w: off + (idx + 1) * w]

            lfwdT = c128s('lfwdT')
            linvT = c128s('linvT')
            negIT = c128s('negIT')

            xqs = {}

            def fetch_x(q):
                xq = xqpool.tile([128, THW], F16, tag="xq")
                nc.sync.dma_start(xq[:, :], x16d.ap()[q])
                xqs[q] = xq

            # ---- per-q chunk builders; st holds per-q tiles ----
            def h1_chunks(q, st):
                """front half: LIF1 (scaled, leak-free), ST1, colMMs, ST2"""
                def pre():
                    fetch_x(q + 1) if q + 1 < NQ else None
                    st['xq'] = xqs[q]
                    st['u1h'] = spool.tile([128, HW], F16, tag="u1h", name=f"u1h{q}")
                    nc.vector.memset(st['u1h'][:, :], 0.0)
                    st['sN'] = bigpool.tile([128, THW], F16, tag="sN", name=f"sN{q}")
                    st['sps'] = []

                def lif1(t):
                    u1h = st['u1h']
                    thr = float(2.0 ** t)
                    nc.vector.tensor_tensor(
                        u1h[:, :], u1h[:, :],
                        st['xq'][:, t * HW:(t + 1) * HW], ALU.add)
                    sp = sppool.tile([128, 34 * 34], F8, tag="sp")
                    if q < 3:
                        # 12 pool bufs; interiors always overwritten,
                        # borders must be zero once per buf
                        nc.gpsimd.memset(sp[:, :], 0.0)
                    sp3 = sp[:, :].rearrange("p (h w) -> p h w", h=34,
                                             w=34)
                    sNt = st['sN'][:, t * HW:(t + 1) * HW]
                    nc.vector.tensor_scalar(sNt, u1h[:, :], thr,
                                            None, ALU.is_ge)
                    ns = wpool.tile([128, HW], F16, tag="ns")
                    nc.vector.tensor_scalar(ns[:, :], u1h[:, :], thr,
                                            None, ALU.is_lt)
                    if t < T - 1:
                        nc.vector.tensor_tensor(u1h[:, :], u1h[:, :],
                                                ns[:, :], ALU.mult)
                    # fp8 padded conv tile built from flat spikes (Pool)
                    nc.gpsimd.tensor_copy(
                        sp3[:, 1:33, 1:33],
                        sNt.rearrange("p (h w) -> p h w", h=32, w=32))
                    st['sps'].append(sp3)

                def st1ch():
                    st['sD'] = bigpool.tile([128, THW], F16, tag="sD", name=f"sD{q}")
                    nc.vector.transpose(st['sD'][:, :], st['sN'][:, :])
                    st['M1s'] = bigpool.tile([128, THW], F16, tag="M1s", name=f"M1s{q}")

                def colmm(t):
                    M1 = psScr.tile([128, HW], F32, tag="scr")
                    sD3 = st['sD'][:, t * HW:(t + 1) * HW].rearrange(
                        "p (h cc) -> p h cc", h=32, cc=32)
                    for j in (0, 1):
                        nc.tensor.matmul(
                            M1[:, 512 * j:512 * j + 512].rearrange(
                                "p (cc h) -> p h cc", cc=16, h=32),
                            lfwdT, sD3[:, :, 16 * j:16 * j + 16],
                            start=True, stop=True)
                    nc.scalar.copy(st['M1s'][:, t * HW:(t + 1) * HW],
                                   M1[:, :])

                def st2ch():
                    st['sB'] = bigpool.tile([128, THW], F16, tag="sB", name=f"sB{q}")
                    nc.vector.transpose(st['sB'][:, :], st['M1s'][:, :])

                def first(t=0):
                    pre()
                    lif1(0)
                return ([first] + [lambda t=t: lif1(t) for t in (1, 2, 3)]
                        + [st1ch]
                        + [lambda t=t: colmm(t) for t in range(T)]
                        + [st2ch])

            def h2_chunks(q, st):
                """back half: negIF1+LIF2, inverse haar, mix+negIF2+out"""
                inv0cF = c128s('inv0cF', q, HW)
                thr2F = c128s('thr2F', q, HW)
                negBF = c128s('negBF', q, HW)
                mixTq = c128s('mixT', q)
                a2dTq = c128s('a2dT', q)

                def bpre():
                    st['V1'] = psSt.tile([128, HW], F32, tag="V1", name=f"V1{q}")
                    st['u2h'] = spool.tile([128, HW], F16, tag="u2h", name=f"u2h{q}")
                    nc.vector.tensor_copy(st['u2h'][:, :], negBF)
                    st['s2a'] = bigpool.tile([128, THW], F16, tag="s2a", name=f"s2a{q}")

                def bt(t):
                    V1 = st['V1']
                    u2h = st['u2h']
                    for h_ in (0, 512):
                        nc.tensor.matmul(
                            V1[:, h_:h_ + 512], lfwdT,
                            st['sB'][:, t * HW + h_:t * HW + h_ + 512],
                            start=(t == 0), stop=(t == T - 1),
                            skip_group_check=True)
                    g1 = wpool.tile([128, HW], F16, tag="g1")
                    g2 = wpool.tile([128, HW], F16, tag="g2")
                    nc.scalar.activation(g1[:, :], V1[:, :], AF.Sign,
                                         bias=one_m1[:, :])
                    nc.scalar.activation(g2[:, :], V1[:, :], AF.Sign,
                                         bias=one_p1[:, :])
                    st1 = wpool.tile([128, HW], F16, tag="st1")
                    nc.gpsimd.tensor_tensor(st1[:, :], g1[:, :], g2[:, :],
                                            ALU.add)
                    if t < T - 1:
                        for h_ in (0, 512):
                            nc.tensor.matmul(V1[:, h_:h_ + 512], negIT,
                                             st1[:, h_:h_ + 512],
                                             start=False, stop=False,
                                             skip_group_check=True)
                    pprod = wpool.tile([128, HW], F16, tag="pprod")
                    nc.vector.tensor_tensor(pprod[:, :], st1[:, :],
                                            inv0cF, ALU.mult)
                    nc.vector.tensor_scalar(u2h[:, :], u2h[:, :], 0.5,
                                            None, ALU.mult)
                    nc.vector.tensor_tensor(u2h[:, :], u2h[:, :],
                                            pprod[:, :], ALU.add)
                    s2t = st['s2a'][:, t * HW:(t + 1) * HW]
                    nc.vector.tensor_tensor(s2t, u2h[:, :], thr2F,
                                            ALU.is_ge)
                    nc.vector.copy_predicated(u2h[:, :],
                                              s2t.bitcast(U16), negBF)

                def zpre():
                    st['Zs'] = bigpool.tile([128, THW], F16, tag="Zs", name=f"Zs{q}")

                def zrow(t):
                    Z = psScr.tile([128, HW], F32, tag="scr")
                    for h_ in (0, 512):
                        nc.tensor.matmul(
                            Z[:, h_:h_ + 512], linvT,
                            st['s2a'][:, t * HW + h_:t * HW + h_ + 512],
                            start=True, stop=True)
                    nc.vector.tensor_copy(
                        st['Zs'][:, t * HW:(t + 1) * HW], Z[:, :])

                def st3ch():
                    st['ZT'] = bigpool.tile([128, THW], F16, tag="ZT", name=f"ZT{q}")
                    nc.vector.transpose(st['ZT'][:, :], st['Zs'][:, :])
                    st['Ws'] = bigpool.tile([128, THW], F16, tag="Ws", name=f"Ws{q}")

                def wcol(t):
                    W2 = psScr.tile([128, HW], F32, tag="scr")
                    ZT3 = st['ZT'][:, t * HW:(t + 1) * HW].rearrange(
                        "p (cc a) -> p cc a", cc=32, a=32)
                    for j in (0, 1):
                        nc.tensor.matmul(
                            W2[:, 512 * j:512 * j + 512].rearrange(
                                "p (a cc) -> p cc a", a=16, cc=32),
                            linvT, ZT3[:, :, 16 * j:16 * j + 16],
                            start=True, stop=True)
                    nc.scalar.copy(st['Ws'][:, t * HW:(t + 1) * HW],
                                   W2[:, :])

                def st4ch():
                    st['haarA'] = bigpool.tile([128, THW], F16, tag="haarA", name=f"haarA{q}")
                    nc.vector.transpose(st['haarA'][:, :], st['Ws'][:, :])
                    st['v2'] = psSt.tile([128, HW], F32, tag="v2", name=f"v2{q}")
                    st['st2a'] = bigpool.tile([128, THW], F16, tag="st2a", name=f"st2a{q}")

                def dt(t):
                    v2 = st['v2']
                    st2a = st['st2a']
                    for h_ in (0, 512):
                        nc.tensor.matmul(v2[:, h_:h_ + 512], mixTq,
                                         st['haarA'][:, t * HW + h_:
                                                     t * HW + h_ + 512],
                                         start=(t == 0), stop=False,
                                         skip_group_check=True)
                        nc.tensor.matmul(v2[:, h_:h_ + 512],
                                         c1s('beta1', q, 128),
                                         c1s('p0row')[0:1, h_:h_ + 512],
                                         start=False, stop=(t == T - 1),
                                         skip_group_check=True)
                    g1b = wpool.tile([128, HW], F16, tag="g1b")
                    g2b = wpool.tile([128, HW], F16, tag="g2b")
                    nc.scalar.activation(g1b[:, :], v2[:, :], AF.Sign,
                                         bias=one_m1[:, :])
                    nc.scalar.activation(g2b[:, :], v2[:, :], AF.Sign,
                                         bias=one_p1[:, :])
                    st2t = st2a[:, t * HW:(t + 1) * HW]
                    nc.gpsimd.tensor_tensor(st2t, g1b[:, :], g2b[:, :],
                                            ALU.add)
                    if t < T - 1:
                        for h_ in (0, 512):
                            nc.tensor.matmul(v2[:, h_:h_ + 512], negIT,
                                             st2a[:, t * HW + h_:
                                                  t * HW + h_ + 512],
                                             start=False, stop=False,
                                             skip_group_check=True)

                    OUT = psScr.tile([128, HW], F32, tag="scr")
                    sp3 = st['sps'][t]
                    for ti in range(9):
                        dy, dx = TAPS[ti]
                        cT = c128s('convT', q * 9 + ti)
                        rhs = sp3[:, 1 + dy:33 + dy, 1 + dx:33 + dx]
                        nc.tensor.matmul(OUT[:, 0:512], cT,
                                         rhs[:, 0:16, :],
                                         start=(ti == 0), stop=False,
                                         skip_group_check=True)
                        nc.tensor.matmul(OUT[:, 512:1024], cT,
                                         rhs[:, 16:32, :],
                                         start=(ti == 0), stop=False,
                                         skip_group_check=True)
                    xqd = xqdpool.tile([128, HW], F16, tag="xqd")
                    nc.sync.dma_start(
                        xqd[:, :],
                        x16d.ap()[q][:, t * HW:(t + 1) * HW])
                    for h_ in (0, 512):
                        nc.tensor.matmul(OUT[:, h_:h_ + 512], a2dTq,
                                         st2a[:, t * HW + h_:
                                              t * HW + h_ + 512],
                                         start=False, stop=False,
                                         skip_group_check=True)
                        nc.tensor.matmul(OUT[:, h_:h_ + 512],
                                         c128s('idI', t),
                                         xqd[:, h_:h_ + 512],
                                         start=False, stop=True,
                                         skip_group_check=True)
                    osb = ostpool.tile([128, HW], F16, tag="ost")
                    nc.scalar.activation(osb[:, :], OUT[:, :],
                                         AF.Identity,
                                         bias=betaA[:, q:q + 1])
                    nc.sync.dma_start(
                        outd.ap()[q][:, t * HW:(t + 1) * HW], osb[:, :])

                def bfirst(t=0):
                    bpre()
                    bt(0)

                def z0(t=0):
                    zpre()
                    zrow(0)
                return ([bfirst] + [lambda t=t: bt(t) for t in (1, 2, 3)]
                        + [z0] + [lambda t=t: zrow(t) for t in (1, 2, 3)]
                        + [st3ch]
                        + [lambda t=t: wcol(t) for t in range(T)]
                        + [st4ch]
                        + [lambda t=t: dt(t) for t in range(T)])

            # ---- rolling pipelined emission across q ----------------
            # q's chunk list L(q) has 27 entries; q+1 starts OFF chunks
            # after q. Smaller OFF = deeper overlap; floor ~14 set by
            # xq triple-buffering.
            OFF = int(os.environ.get('FATM_OFF', '9'))
            fetch_x(0)
            sts = [dict() for _ in range(NQ)]
            Ls = [h1_chunks(q, sts[q]) + h2_chunks(q, sts[q])
                  for q in range(NQ)]
            nch = len(Ls[0])
            for s in range((NQ - 1) * OFF + nch):
                for q in range(NQ):
                    c = s - q * OFF
                    if 0 <= c < nch:
                        Ls[q][c]()

    _split_excess_waits(nc)
    return nc


_NC_CACHE = None


def _get_nc():
    global _NC_CACHE
    if _NC_CACHE is None:
        _NC_CACHE = _build_program()
    return _NC_CACHE


def _prep_inputs(inputs):
    x = np.asarray(inputs['x'], np.float32)          # [T, B, C, H, W]
    c128, c1, betaAc = _host_consts(inputs)
    # x16[t] = 2^(t-1) * x_t: leak-free scaled LIF1 state form
    tscale = (2.0 ** (np.arange(T) - 1)).astype(np.float32)
    in_maps = []
    for b in range(NCORES):
        xb = (tscale[:, None, None, None] * x[:, b].reshape(T, C, Hh, Ww)) \
            .reshape(T, NQ, 128, HW).transpose(1, 2, 0, 3)
        m = {'x16h': np.ascontiguousarray(xb).reshape(NQ, 128, THW)
             .astype(np.float16),
             'c128': c128, 'c1': c1, 'betaA': betaAc}
        in_maps.append(m)
    return in_maps


def kernel(**inputs):
    in_maps = _prep_inputs(inputs)
    nc = _get_nc()
    res = run_bass_kernel_spmd(nc, in_maps, list(range(NCORES))).results
    # out [NQ, 128, T*HW] f16 -> [T, B, C, H, W] f32
    outs = []
    for b in range(NCORES):
        ob = res[b]['out'].astype(np.float32).reshape(NQ, 128, T, HW) \
            .transpose(2, 0, 1, 3)
        outs.append(ob.reshape(T, C, HW))
    out = np.stack(outs, axis=1)
    return out.reshape(T, Bb, C, Hh, Ww).astype(np.float32)
